# revision 10
# baseline (speedup 1.0000x reference)
"""Bass/Tile kernel for causal self-attention, head-sharded across 8 cores.

Per-core layout (core c owns heads 2c, 2c+1):
  xb    [C, B*T]         bf16   x transposed (feature-major), same on all cores
  wqk   [128, KC, 2, 128] bf16  W_qkv q/k column-slices; fc0=q, fc1=k,
                                cols = [h0 d0-63 | h1 d0-63]
  wv    [128, KC, 128]   bf16   W_qkv v column-slice
  wp    [128, C]         bf16   W_proj row-slice (this core's 128 head dims)
  bq    [128, 1]         f32    q bias (k bias is absorbed: softmax is
                                invariant to per-query logit shifts, and
                                (q+bq)@k reproduces the only surviving term;
                                v bias commutes to host as W_proj^T @ b_v)
  pbias [128, B, T/128]  f32    per-key bias: 0 or -1e30
  mtri  [128, 2, 128]    bf16   lower-triangular causal mask
  outT  [C, B*T]         bf16   partial projection output (pre-bias)

Compute:
  1) QKV projection in bf16 -> qT (bias-added) / kT tiles;
     V^T -> DMA-transpose -> token-major V bf16.
  2) per (b, q-block): key chunks processed diagonal-first in descending
     offset, each clipped to its causal query range [o, 512); the newly
     opened query range gets start=True on the psO/psD accumulation.
     S^T bf16 per chunk, P = exp(scale*S + pbias) bf16 on ACT, triangular
     mask mul (first 128 cols only) on GpSimd, PV + denominator bf16 with
     two heads column-packed via tile_position.
  3) A = O * recip(denom); inline bf16 output projection; bf16 DMA out.
"""

import concourse.bass as bass
import concourse.mybir as mybir
import concourse.tile as tile
from concourse import bacc

F32 = mybir.dt.float32
BF16 = mybir.dt.bfloat16
F8 = mybir.dt.float8e4
AF = mybir.ActivationFunctionType
DR = mybir.MatmulPerfMode.DoubleRow

SHIFT = 0.0
_NOCLIP = False


def build_nc(B=4, T=2048, C=1024, HPC=2, D=64, TB=512, num_devices=8,
             scale=None):
    if scale is None:
        scale = D ** -0.5
    NT = B * T
    NB = NT // TB              # 512-token blocks (16)
    BPB = T // TB              # blocks per batch (4)
    CPB = TB // 128            # 128-key chunks per block (4)
    NCH = T // 128             # key chunks per batch (16)
    KC = C // 128              # contraction chunks (8)
    assert HPC == 2 and HPC * D == 128 and KC % 2 == 0 and CPB == 4

    nc = bacc.Bacc("TRN2", target_bir_lowering=False, debug=False,
                   num_devices=num_devices)

    xb = nc.dram_tensor("xb", [C, NT], BF16, kind="ExternalInput")
    wqk = nc.dram_tensor("wqk", [128, KC, 2, 128], BF16, kind="ExternalInput")
    wv = nc.dram_tensor("wv", [128, KC, 128], BF16, kind="ExternalInput")
    wp = nc.dram_tensor("wp", [128, C], BF16, kind="ExternalInput")
    bq = nc.dram_tensor("bq", [128, 1], F32, kind="ExternalInput")
    pbias = nc.dram_tensor("pbias", [128, B, NCH], F32, kind="ExternalInput")
    mtri = nc.dram_tensor("mtri", [128, 2, 128], BF16, kind="ExternalInput")
    outT = nc.dram_tensor("outT", [C, NT], BF16, kind="ExternalOutput")

    with tile.TileContext(nc) as tc:
        with (
            tc.tile_pool(name="const", bufs=1) as const,
            tc.tile_pool(name="persist", bufs=1) as persist,
            tc.tile_pool(name="xp", bufs=2) as xp,
            tc.tile_pool(name="vp", bufs=3) as vp,
            tc.tile_pool(name="pp", bufs=6) as pp,
            tc.tile_pool(name="rp", bufs=2) as rp,
            tc.tile_pool(name="op", bufs=10) as op,
            tc.tile_pool(name="psmm", bufs=4, space="PSUM") as psmm,
            tc.tile_pool(name="pss", bufs=2, space="PSUM") as pss,
        ):
            # ---- constants ----
            wqk_sb = const.tile([128, KC, 2, 128], BF16, tag="wqk", name="wqk_sb")
            nc.sync.dma_start(out=wqk_sb[:], in_=wqk[:])
            wv_sb = const.tile([128, KC, 128], BF16, tag="wv", name="wv_sb")
            nc.sync.dma_start(out=wv_sb[:], in_=wv[:])
            wp_sb = const.tile([128, C], BF16, tag="wp", name="wp_sb")
            nc.scalar.dma_start(out=wp_sb[:], in_=wp[:])
            bq_sb = const.tile([128, 1], F32, tag="bq", name="bq_sb")
            nc.sync.dma_start(out=bq_sb[:], in_=bq[:])
            pb_sb = const.tile([128, B, NCH], F32, tag="pb", name="pb_sb")
            nc.sync.dma_start(out=pb_sb[:], in_=pbias[:])
            mk_sb = const.tile([128, 2, 128], BF16, tag="mk", name="mk_sb")
            nc.scalar.dma_start(out=mk_sb[:], in_=mtri[:])
            ones_sb = const.tile([128, 64], BF16, tag="ones", name="ones_sb")
            nc.vector.memset(ones_sb[:], 1.0)

            # ---- persistent per-block tiles ----
            qT = [persist.tile([128, TB], BF16, tag=f"qT{i}", name=f"qT{i}")
                  for i in range(NB)]
            kT = [persist.tile([128, TB], BF16, tag=f"kT{i}", name=f"kT{i}")
                  for i in range(NB)]
            V = [persist.tile([128, CPB, 128], BF16, tag=f"V{i}", name=f"V{i}")
                  for i in range(NB)]

            GRP = 4
            W = GRP * TB

            def qkv_group(g, split=4):
                xt = xp.tile([128, KC, W], BF16, tag="xt", name="xt")
                # split loads across the two HWDGE queues (sync + scalar)
                for kc in range(KC):
                    step = W // split
                    for s in range(split):
                        eng = nc.sync if (kc * split + s) % 2 == 0 else nc.scalar
                        eng.dma_start(
                            out=xt[:, kc, s * step:(s + 1) * step],
                            in_=xb[kc * 128:(kc + 1) * 128,
                                   g * W + s * step:g * W + (s + 1) * step])
                for tl in range(GRP):
                    tb = g * GRP + tl
                    ts = slice(tl * TB, (tl + 1) * TB)
                    for fc in range(2):
                        ps = psmm.tile([128, TB], F32, tag="ps", name="ps")
                        for kc in range(KC):
                            nc.tensor.matmul(
                                ps[:], lhsT=wqk_sb[:, kc, fc, :],
                                rhs=xt[:, kc, ts],
                                start=(kc == 0), stop=(kc == KC - 1))
                        if fc == 0:
                            nc.vector.tensor_scalar_add(
                                out=qT[tb][:], in0=ps[:], scalar1=bq_sb[:])
                        else:
                            nc.vector.tensor_copy(kT[tb][:], ps[:])
                    # V^T then transpose to token-major fp8
                    ps = psmm.tile([128, TB], F32, tag="ps", name="ps")
                    for kc in range(KC):
                        nc.tensor.matmul(
                            ps[:], lhsT=wv_sb[:, kc, :],
                            rhs=xt[:, kc, ts],
                            start=(kc == 0), stop=(kc == KC - 1))
                    vs = vp.tile([128, TB], BF16, tag="vs", name="vs")
                    nc.vector.tensor_copy(vs[:], ps[:])
                    nc.sync.dma_start_transpose(out=V[tb][:], in_=vs[:])

            # ---- attention + inline projection ----
            def attn_block(b, qb, out_eng):
                gb = b * BPB + qb
                diag0 = qb * CPB
                nchunks = diag0 + CPB
                psO = psmm.tile([128, TB], F32, tag="ps", name="psO")
                psD = psmm.tile([128, TB], F32, tag="ps", name="psD")
                for ci in range(nchunks):
                    blk = b * BPB + ci // CPB
                    cl = ci % CPB
                    is_diag = ci >= diag0
                    o = (ci - diag0) * 128 if (is_diag and not _NOCLIP) else 0
                    Wc = TB - o
                    start = (ci == 0)
                    last = (ci == nchunks - 1)
                    psS = pss.tile([128, 2, TB], F32, tag="psS", name="psS")
                    for h in range(HPC):
                        nc.tensor.matmul(
                            psS[:, h, 0:Wc],
                            lhsT=kT[blk][h * 64:(h + 1) * 64,
                                         cl * 128:(cl + 1) * 128],
                            rhs=qT[gb][h * 64:(h + 1) * 64, o:TB],
                            start=True, stop=True)
                    pt = pp.tile([128, 2, TB], BF16, tag="pt", name="pt")
                    nc.scalar.activation(
                        out=pt[:, :, 0:Wc], in_=psS[:, :, 0:Wc],
                        func=AF.Exp, bias=pb_sb[:, b, ci:ci + 1], scale=scale)
                    if is_diag:
                        nc.gpsimd.tensor_mul(
                            pt[:, :, 0:128], pt[:, :, 0:128], mk_sb[:])
                    for h in range(HPC):
                        nc.tensor.matmul(
                            psO[h * 64:(h + 1) * 64, o:TB],
                            lhsT=V[blk][:, cl, h * 64:(h + 1) * 64],
                            rhs=pt[:, h, 0:Wc],
                            start=start, stop=last,
                            tile_position=(0, h * 64))
                    for h in range(HPC):
                        nc.tensor.matmul(
                            psD[h * 64:(h + 1) * 64, o:TB],
                            lhsT=ones_sb[:],
                            rhs=pt[:, h, 0:Wc],
                            start=start, stop=last,
                            tile_position=(0, h * 64))
                # normalize: A^T = O^T * (1/denom)
                rt = rp.tile([128, TB], F32, tag="rt", name="rt")
                nc.vector.reciprocal_approx_fast(out=rt[:], in_=psD[:])
                at = pp.tile([128, TB], BF16, tag="at", name="at")
                nc.vector.tensor_mul(at[:], psO[:], rt[:])
                # inline output projection
                for fc in range(C // 128):
                    ps = psmm.tile([128, TB], F32, tag="ps", name="ps")
                    nc.tensor.matmul(ps[:],
                                     lhsT=wp_sb[:, fc * 128:(fc + 1) * 128],
                                     rhs=at[:], start=True, stop=True)
                    ot = op.tile([128, TB], BF16, tag="ot", name="ot")
                    nc.vector.tensor_copy(ot[:], ps[:])
                    out_eng.dma_start(
                        out=outT[fc * 128:(fc + 1) * 128,
                                 gb * TB:(gb + 1) * TB],
                        in_=ot[:])

            # emission: qkv group g covers batch g; weave attention between
            qkv_group(0, split=8)
            qkv_group(1)
            for qb in range(BPB):
                attn_block(0, qb, nc.sync)
            qkv_group(2)
            for qb in range(BPB):
                attn_block(1, qb, nc.scalar)
            qkv_group(3)
            for qb in range(BPB):
                attn_block(2, qb, nc.sync)
            for qb in reversed(range(BPB)):
                attn_block(3, qb, nc.scalar)

    nc.compile()
    return nc


def prep_core_inputs(x, key_padding_mask, W_qkv, b_qkv, W_proj,
                     n_cores=8, TB=512):
    """Host-side sharding: build the per-core input maps."""
    import numpy as np
    import ml_dtypes

    BFD = ml_dtypes.bfloat16
    B, T, C = x.shape
    D = 64
    H = C // D
    HPC = H // n_cores
    BT = B * T
    KC = C // 128
    NCH = T // 128

    xb = np.ascontiguousarray(x.reshape(BT, C).T).astype(BFD)       # [C, BT]

    pb = np.where(key_padding_mask, np.float32(-1e30),
                  np.float32(0.0)).astype(np.float32)
    pb = np.ascontiguousarray(
        pb.reshape(B, NCH, 128).transpose(2, 0, 1)).astype(np.float32)

    p = np.arange(128)[:, None]
    j = np.arange(128)[None, :]
    tri = (p <= j).astype(np.float32)                               # [128,128]
    mtri = np.repeat(tri[:, None, :], 2, axis=1).astype(ml_dtypes.bfloat16)

    in_maps = []
    for c in range(n_cores):
        hs = [HPC * c + i for i in range(HPC)]
        qcols = np.concatenate([h * D + np.arange(D) for h in hs])
        wq = W_qkv[:, qcols]                                        # [C,128]
        wk = W_qkv[:, C + qcols]
        wv = W_qkv[:, 2 * C + qcols]
        wqkc = np.stack([wq.reshape(KC, 128, 128),
                         wk.reshape(KC, 128, 128)], axis=2)         # [KC,128,2,128]
        wqkc = np.ascontiguousarray(wqkc.transpose(1, 0, 2, 3)).astype(BFD)
        wvc = np.ascontiguousarray(
            wv.reshape(KC, 128, 128).transpose(1, 0, 2)).astype(BFD)
        bqc = np.ascontiguousarray(
            b_qkv[qcols].reshape(128, 1)).astype(np.float32)
        wpc = np.ascontiguousarray(
            W_proj[qcols, :]).astype(ml_dtypes.bfloat16)
        in_maps.append({
            "xb": xb, "wqk": wqkc, "wv": wvc, "wp": wpc,
            "bq": bqc, "pbias": pb, "mtri": mtri,
        })
    return in_maps


def combine_outputs(results, B, T, C, b_qkv, b_proj, W_proj):
    import numpy as np
    acc = np.zeros((C, B * T), np.float32)
    for r in results:
        acc += np.asarray(r["outT"]).astype(np.float32)
    b_v = np.asarray(b_qkv, np.float32)[2 * C:3 * C]
    b_out = np.asarray(b_proj, np.float32) + \
        np.asarray(W_proj, np.float32).T @ b_v
    out = acc.T.reshape(B, T, C) + b_out
    return out.astype(np.float32)


# ---------------------------------------------------------------------------
# Self-contained entry point for the grading harness.
# ---------------------------------------------------------------------------
import numpy as np

_NC_CACHE = {}


def _get_nc():
    if "nc" not in _NC_CACHE:
        _NC_CACHE["nc"] = build_nc(B=4, T=2048, C=1024, num_devices=8)
    return _NC_CACHE["nc"]


def kernel(x, key_padding_mask, W_qkv, b_qkv, W_proj, b_proj):
    from concourse.bass_utils import run_bass_kernel_spmd

    x = np.asarray(x, dtype=np.float32)
    key_padding_mask = np.asarray(key_padding_mask).astype(bool)
    W_qkv = np.asarray(W_qkv, dtype=np.float32)
    b_qkv = np.asarray(b_qkv, dtype=np.float32)
    W_proj = np.asarray(W_proj, dtype=np.float32)
    b_proj = np.asarray(b_proj, dtype=np.float32)

    B, T, C = x.shape
    nc = _get_nc()
    in_maps = prep_core_inputs(x, key_padding_mask, W_qkv, b_qkv, W_proj,
                               n_cores=8)
    res = run_bass_kernel_spmd(nc, in_maps, list(range(8)))
    return combine_outputs(res.results, B, T, C, b_qkv, b_proj, W_proj)


# revision 11
# speedup vs baseline: 1.0166x; 1.0166x over previous
"""Bass/Tile kernel for causal self-attention, head-sharded across 8 cores.

Per-core layout (core c owns heads 2c, 2c+1):
  xb    [C, B*T]         bf16   x transposed (feature-major), same on all cores
  wqk   [128, KC, 2, 128] bf16  W_qkv q/k column-slices; fc0=q, fc1=k,
                                cols = [h0 d0-63 | h1 d0-63]
  wv    [128, KC, 128]   bf16   W_qkv v column-slice
  wp    [128, C]         bf16   W_proj row-slice (this core's 128 head dims)
  bq    [128, 1]         f32    q bias (k bias is absorbed: softmax is
                                invariant to per-query logit shifts, and
                                (q+bq)@k reproduces the only surviving term;
                                v bias commutes to host as W_proj^T @ b_v)
  pbias [128, B, T/128]  f32    per-key bias: 0 or -1e30
  mtri  [128, 2, 128]    bf16   lower-triangular causal mask
  outT  [C, B*T]         bf16   partial projection output (pre-bias)

Compute:
  1) QKV projection in bf16 -> qT (bias-added) / kT tiles;
     V^T -> DMA-transpose -> token-major V bf16.
  2) per (b, q-block): key chunks processed diagonal-first in descending
     offset, each clipped to its causal query range [o, 512); the newly
     opened query range gets start=True on the psO/psD accumulation.
     S^T bf16 per chunk, P = exp(scale*S + pbias) bf16 on ACT, triangular
     mask mul (first 128 cols only) on GpSimd, PV + denominator bf16 with
     two heads column-packed via tile_position.
  3) A = O * recip(denom); inline bf16 output projection; bf16 DMA out.
"""

import concourse.bass as bass
import concourse.mybir as mybir
import concourse.tile as tile
from concourse import bacc

F32 = mybir.dt.float32
BF16 = mybir.dt.bfloat16
F8 = mybir.dt.float8e4
AF = mybir.ActivationFunctionType
DR = mybir.MatmulPerfMode.DoubleRow

SHIFT = 0.0
_NOCLIP = False


def build_nc(B=4, T=2048, C=1024, HPC=2, D=64, TB=512, num_devices=8,
             scale=None):
    if scale is None:
        scale = D ** -0.5
    NT = B * T
    NB = NT // TB              # 512-token blocks (16)
    BPB = T // TB              # blocks per batch (4)
    CPB = TB // 128            # 128-key chunks per block (4)
    NCH = T // 128             # key chunks per batch (16)
    KC = C // 128              # contraction chunks (8)
    assert HPC == 2 and HPC * D == 128 and KC % 2 == 0 and CPB == 4

    nc = bacc.Bacc("TRN2", target_bir_lowering=False, debug=False,
                   num_devices=num_devices)

    xb = nc.dram_tensor("xb", [C, NT], BF16, kind="ExternalInput")
    wqk = nc.dram_tensor("wqk", [128, KC, 2, 128], BF16, kind="ExternalInput")
    wv = nc.dram_tensor("wv", [128, KC, 128], BF16, kind="ExternalInput")
    wp = nc.dram_tensor("wp", [128, C], BF16, kind="ExternalInput")
    bq = nc.dram_tensor("bq", [128, 1], F32, kind="ExternalInput")
    pbias = nc.dram_tensor("pbias", [128, B, NCH], F32, kind="ExternalInput")
    mtri = nc.dram_tensor("mtri", [128, 2, 128], BF16, kind="ExternalInput")
    outT = nc.dram_tensor("outT", [C, NT], BF16, kind="ExternalOutput")

    with tile.TileContext(nc) as tc:
        with (
            tc.tile_pool(name="const", bufs=1) as const,
            tc.tile_pool(name="persist", bufs=1) as persist,
            tc.tile_pool(name="xp", bufs=2) as xp,
            tc.tile_pool(name="vp", bufs=3) as vp,
            tc.tile_pool(name="pp", bufs=6) as pp,
            tc.tile_pool(name="rp", bufs=2) as rp,
            tc.tile_pool(name="op", bufs=10) as op,
            tc.tile_pool(name="psmm", bufs=2, space="PSUM") as psmm,
            tc.tile_pool(name="psa", bufs=2, space="PSUM") as psa,
            tc.tile_pool(name="pss", bufs=2, space="PSUM") as pss,
        ):
            # ---- constants ----
            wqk_sb = const.tile([128, KC, 2, 128], BF16, tag="wqk", name="wqk_sb")
            nc.sync.dma_start(out=wqk_sb[:], in_=wqk[:])
            wv_sb = const.tile([128, KC, 128], BF16, tag="wv", name="wv_sb")
            nc.sync.dma_start(out=wv_sb[:], in_=wv[:])
            wp_sb = const.tile([128, C], BF16, tag="wp", name="wp_sb")
            nc.scalar.dma_start(out=wp_sb[:], in_=wp[:])
            bq_sb = const.tile([128, 1], F32, tag="bq", name="bq_sb")
            nc.sync.dma_start(out=bq_sb[:], in_=bq[:])
            pb_sb = const.tile([128, B, NCH], F32, tag="pb", name="pb_sb")
            nc.sync.dma_start(out=pb_sb[:], in_=pbias[:])
            mk_sb = const.tile([128, 2, 128], BF16, tag="mk", name="mk_sb")
            nc.scalar.dma_start(out=mk_sb[:], in_=mtri[:])
            ones_sb = const.tile([128, 64], BF16, tag="ones", name="ones_sb")
            nc.vector.memset(ones_sb[:], 1.0)

            # ---- persistent per-block tiles ----
            qT = [persist.tile([128, TB], BF16, tag=f"qT{i}", name=f"qT{i}")
                  for i in range(NB)]
            kT = [persist.tile([128, TB], BF16, tag=f"kT{i}", name=f"kT{i}")
                  for i in range(NB)]
            V = [persist.tile([128, CPB, 128], BF16, tag=f"V{i}", name=f"V{i}")
                  for i in range(NB)]

            GRP = 4
            W = GRP * TB

            def qkv_group(g, split=4):
                xt = xp.tile([128, KC, W], BF16, tag="xt", name="xt")
                # split loads across the two HWDGE queues (sync + scalar)
                for kc in range(KC):
                    step = W // split
                    for s in range(split):
                        eng = nc.sync if (kc * split + s) % 2 == 0 else nc.scalar
                        eng.dma_start(
                            out=xt[:, kc, s * step:(s + 1) * step],
                            in_=xb[kc * 128:(kc + 1) * 128,
                                   g * W + s * step:g * W + (s + 1) * step])
                for tl in range(GRP):
                    tb = g * GRP + tl
                    ts = slice(tl * TB, (tl + 1) * TB)
                    for fc in range(2):
                        ps = psmm.tile([128, TB], F32, tag="ps", name="ps")
                        for kc in range(KC):
                            nc.tensor.matmul(
                                ps[:], lhsT=wqk_sb[:, kc, fc, :],
                                rhs=xt[:, kc, ts],
                                start=(kc == 0), stop=(kc == KC - 1))
                        if fc == 0:
                            nc.vector.tensor_scalar_add(
                                out=qT[tb][:], in0=ps[:], scalar1=bq_sb[:])
                        else:
                            nc.vector.tensor_copy(kT[tb][:], ps[:])
                    # V^T then transpose to token-major fp8
                    ps = psmm.tile([128, TB], F32, tag="ps", name="ps")
                    for kc in range(KC):
                        nc.tensor.matmul(
                            ps[:], lhsT=wv_sb[:, kc, :],
                            rhs=xt[:, kc, ts],
                            start=(kc == 0), stop=(kc == KC - 1))
                    vs = vp.tile([128, TB], BF16, tag="vs", name="vs")
                    nc.vector.tensor_copy(vs[:], ps[:])
                    nc.sync.dma_start_transpose(out=V[tb][:], in_=vs[:])

            # ---- attention + inline projection ----
            def attn_block(b, qb, out_eng):
                gb = b * BPB + qb
                diag0 = qb * CPB
                nchunks = diag0 + CPB
                psO = psa.tile([128, TB], F32, tag="psa", name="psO")
                psD = psa.tile([128, TB], F32, tag="psa", name="psD")
                for ci in range(nchunks):
                    blk = b * BPB + ci // CPB
                    cl = ci % CPB
                    is_diag = ci >= diag0
                    o = (ci - diag0) * 128 if (is_diag and not _NOCLIP) else 0
                    Wc = TB - o
                    start = (ci == 0)
                    last = (ci == nchunks - 1)
                    psS = pss.tile([128, 2, TB], F32, tag="psS", name="psS")
                    for h in range(HPC):
                        nc.tensor.matmul(
                            psS[:, h, 0:Wc],
                            lhsT=kT[blk][h * 64:(h + 1) * 64,
                                         cl * 128:(cl + 1) * 128],
                            rhs=qT[gb][h * 64:(h + 1) * 64, o:TB],
                            start=True, stop=True)
                    pt = pp.tile([128, 2, TB], BF16, tag="pt", name="pt")
                    nc.scalar.activation(
                        out=pt[:, :, 0:Wc], in_=psS[:, :, 0:Wc],
                        func=AF.Exp, bias=pb_sb[:, b, ci:ci + 1], scale=scale)
                    if is_diag:
                        nc.vector.tensor_mul(
                            pt[:, :, 0:128], pt[:, :, 0:128], mk_sb[:])
                    for h in range(HPC):
                        nc.tensor.matmul(
                            psO[h * 64:(h + 1) * 64, o:TB],
                            lhsT=V[blk][:, cl, h * 64:(h + 1) * 64],
                            rhs=pt[:, h, 0:Wc],
                            start=start, stop=last,
                            tile_position=(0, h * 64))
                    for h in range(HPC):
                        nc.tensor.matmul(
                            psD[h * 64:(h + 1) * 64, o:TB],
                            lhsT=ones_sb[:],
                            rhs=pt[:, h, 0:Wc],
                            start=start, stop=last,
                            tile_position=(0, h * 64))
                # normalize: A^T = O^T * (1/denom)
                rt = rp.tile([128, TB], F32, tag="rt", name="rt")
                nc.vector.reciprocal_approx_fast(out=rt[:], in_=psD[:])
                at = pp.tile([128, TB], BF16, tag="at", name="at")
                nc.vector.tensor_mul(at[:], psO[:], rt[:])
                # inline output projection
                for fc in range(C // 128):
                    ps = psmm.tile([128, TB], F32, tag="ps", name="ps")
                    nc.tensor.matmul(ps[:],
                                     lhsT=wp_sb[:, fc * 128:(fc + 1) * 128],
                                     rhs=at[:], start=True, stop=True)
                    ot = op.tile([128, TB], BF16, tag="ot", name="ot")
                    nc.vector.tensor_copy(ot[:], ps[:])
                    out_eng.dma_start(
                        out=outT[fc * 128:(fc + 1) * 128,
                                 gb * TB:(gb + 1) * TB],
                        in_=ot[:])

            # emission: qkv group g covers batch g; weave attention between
            qkv_group(0, split=8)
            qkv_group(1)
            qkv_group(2)
            for qb in range(BPB):
                attn_block(0, qb, nc.sync)
            qkv_group(3)
            for qb in range(BPB):
                attn_block(1, qb, nc.scalar)
            for qb in range(BPB):
                attn_block(2, qb, nc.sync)
            for qb in reversed(range(BPB)):
                attn_block(3, qb, nc.scalar)

    nc.compile()
    return nc


def prep_core_inputs(x, key_padding_mask, W_qkv, b_qkv, W_proj,
                     n_cores=8, TB=512):
    """Host-side sharding: build the per-core input maps."""
    import numpy as np
    import ml_dtypes

    BFD = ml_dtypes.bfloat16
    B, T, C = x.shape
    D = 64
    H = C // D
    HPC = H // n_cores
    BT = B * T
    KC = C // 128
    NCH = T // 128

    xb = np.ascontiguousarray(x.reshape(BT, C).T).astype(BFD)       # [C, BT]

    pb = np.where(key_padding_mask, np.float32(-1e30),
                  np.float32(0.0)).astype(np.float32)
    pb = np.ascontiguousarray(
        pb.reshape(B, NCH, 128).transpose(2, 0, 1)).astype(np.float32)

    p = np.arange(128)[:, None]
    j = np.arange(128)[None, :]
    tri = (p <= j).astype(np.float32)                               # [128,128]
    mtri = np.repeat(tri[:, None, :], 2, axis=1).astype(ml_dtypes.bfloat16)

    in_maps = []
    for c in range(n_cores):
        hs = [HPC * c + i for i in range(HPC)]
        qcols = np.concatenate([h * D + np.arange(D) for h in hs])
        wq = W_qkv[:, qcols]                                        # [C,128]
        wk = W_qkv[:, C + qcols]
        wv = W_qkv[:, 2 * C + qcols]
        wqkc = np.stack([wq.reshape(KC, 128, 128),
                         wk.reshape(KC, 128, 128)], axis=2)         # [KC,128,2,128]
        wqkc = np.ascontiguousarray(wqkc.transpose(1, 0, 2, 3)).astype(BFD)
        wvc = np.ascontiguousarray(
            wv.reshape(KC, 128, 128).transpose(1, 0, 2)).astype(BFD)
        bqc = np.ascontiguousarray(
            b_qkv[qcols].reshape(128, 1)).astype(np.float32)
        wpc = np.ascontiguousarray(
            W_proj[qcols, :]).astype(ml_dtypes.bfloat16)
        in_maps.append({
            "xb": xb, "wqk": wqkc, "wv": wvc, "wp": wpc,
            "bq": bqc, "pbias": pb, "mtri": mtri,
        })
    return in_maps


def combine_outputs(results, B, T, C, b_qkv, b_proj, W_proj):
    import numpy as np
    acc = np.zeros((C, B * T), np.float32)
    for r in results:
        acc += np.asarray(r["outT"]).astype(np.float32)
    b_v = np.asarray(b_qkv, np.float32)[2 * C:3 * C]
    b_out = np.asarray(b_proj, np.float32) + \
        np.asarray(W_proj, np.float32).T @ b_v
    out = acc.T.reshape(B, T, C) + b_out
    return out.astype(np.float32)


# ---------------------------------------------------------------------------
# Self-contained entry point for the grading harness.
# ---------------------------------------------------------------------------
import numpy as np

_NC_CACHE = {}


def _get_nc():
    if "nc" not in _NC_CACHE:
        _NC_CACHE["nc"] = build_nc(B=4, T=2048, C=1024, num_devices=8)
    return _NC_CACHE["nc"]


def kernel(x, key_padding_mask, W_qkv, b_qkv, W_proj, b_proj):
    from concourse.bass_utils import run_bass_kernel_spmd

    x = np.asarray(x, dtype=np.float32)
    key_padding_mask = np.asarray(key_padding_mask).astype(bool)
    W_qkv = np.asarray(W_qkv, dtype=np.float32)
    b_qkv = np.asarray(b_qkv, dtype=np.float32)
    W_proj = np.asarray(W_proj, dtype=np.float32)
    b_proj = np.asarray(b_proj, dtype=np.float32)

    B, T, C = x.shape
    nc = _get_nc()
    in_maps = prep_core_inputs(x, key_padding_mask, W_qkv, b_qkv, W_proj,
                               n_cores=8)
    res = run_bass_kernel_spmd(nc, in_maps, list(range(8)))
    return combine_outputs(res.results, B, T, C, b_qkv, b_proj, W_proj)


# revision 12
# speedup vs baseline: 1.0175x; 1.0009x over previous
"""Bass/Tile kernel for causal self-attention, head-sharded across 8 cores.

Per-core layout (core c owns heads 2c, 2c+1):
  xb    [C, B*T]         bf16   x transposed (feature-major), same on all cores
  wqk   [128, KC, 2, 128] bf16  W_qkv q/k column-slices; fc0=q, fc1=k,
                                cols = [h0 d0-63 | h1 d0-63]
  wv    [128, KC, 128]   bf16   W_qkv v column-slice
  wp    [128, C]         bf16   W_proj row-slice (this core's 128 head dims)
  bq    [128, 1]         f32    q bias (k bias is absorbed: softmax is
                                invariant to per-query logit shifts, and
                                (q+bq)@k reproduces the only surviving term;
                                v bias commutes to host as W_proj^T @ b_v)
  pbias [128, B, T/128]  f32    per-key bias: 0 or -1e30
  mtri  [128, 2, 128]    bf16   lower-triangular causal mask
  outT  [C, B*T]         bf16   partial projection output (pre-bias)

Compute:
  1) QKV projection in bf16 -> qT (bias-added) / kT tiles;
     V^T -> DMA-transpose -> token-major V bf16.
  2) per (b, q-block): key chunks processed diagonal-first in descending
     offset, each clipped to its causal query range [o, 512); the newly
     opened query range gets start=True on the psO/psD accumulation.
     S^T bf16 per chunk, P = exp(scale*S + pbias) bf16 on ACT, triangular
     mask mul (first 128 cols only) on GpSimd, PV + denominator bf16 with
     two heads column-packed via tile_position.
  3) A = O * recip(denom); inline bf16 output projection; bf16 DMA out.
"""

import concourse.bass as bass
import concourse.mybir as mybir
import concourse.tile as tile
from concourse import bacc

F32 = mybir.dt.float32
BF16 = mybir.dt.bfloat16
F8 = mybir.dt.float8e4
AF = mybir.ActivationFunctionType
DR = mybir.MatmulPerfMode.DoubleRow

SHIFT = 0.0
_NOCLIP = False


def build_nc(B=4, T=2048, C=1024, HPC=2, D=64, TB=512, num_devices=8,
             scale=None):
    if scale is None:
        scale = D ** -0.5
    NT = B * T
    NB = NT // TB              # 512-token blocks (16)
    BPB = T // TB              # blocks per batch (4)
    CPB = TB // 128            # 128-key chunks per block (4)
    NCH = T // 128             # key chunks per batch (16)
    KC = C // 128              # contraction chunks (8)
    assert HPC == 2 and HPC * D == 128 and KC % 2 == 0 and CPB == 4

    nc = bacc.Bacc("TRN2", target_bir_lowering=False, debug=False,
                   num_devices=num_devices)

    xb = nc.dram_tensor("xb", [C, NT], BF16, kind="ExternalInput")
    wqk = nc.dram_tensor("wqk", [128, KC, 2, 128], BF16, kind="ExternalInput")
    wv = nc.dram_tensor("wv", [128, KC, 128], BF16, kind="ExternalInput")
    wp = nc.dram_tensor("wp", [128, C], BF16, kind="ExternalInput")
    bq = nc.dram_tensor("bq", [128, 1], F32, kind="ExternalInput")
    pbias = nc.dram_tensor("pbias", [128, B, NCH], F32, kind="ExternalInput")
    mtri = nc.dram_tensor("mtri", [128, 2, 128], BF16, kind="ExternalInput")
    outT = nc.dram_tensor("outT", [C, NT], BF16, kind="ExternalOutput")

    with tile.TileContext(nc) as tc:
        with (
            tc.tile_pool(name="const", bufs=1) as const,
            tc.tile_pool(name="persist", bufs=1) as persist,
            tc.tile_pool(name="xp", bufs=2) as xp,
            tc.tile_pool(name="vp", bufs=3) as vp,
            tc.tile_pool(name="pp", bufs=6) as pp,
            tc.tile_pool(name="rp", bufs=2) as rp,
            tc.tile_pool(name="op", bufs=10) as op,
            tc.tile_pool(name="psmm", bufs=2, space="PSUM") as psmm,
            tc.tile_pool(name="psa", bufs=2, space="PSUM") as psa,
            tc.tile_pool(name="pss", bufs=2, space="PSUM") as pss,
        ):
            # ---- constants ----
            wqk_sb = const.tile([128, KC, 2, 128], BF16, tag="wqk", name="wqk_sb")
            nc.sync.dma_start(out=wqk_sb[:], in_=wqk[:])
            wv_sb = const.tile([128, KC, 128], BF16, tag="wv", name="wv_sb")
            nc.sync.dma_start(out=wv_sb[:], in_=wv[:])
            wp_sb = const.tile([128, C], BF16, tag="wp", name="wp_sb")
            nc.scalar.dma_start(out=wp_sb[:], in_=wp[:])
            bq_sb = const.tile([128, 1], F32, tag="bq", name="bq_sb")
            nc.sync.dma_start(out=bq_sb[:], in_=bq[:])
            pb_sb = const.tile([128, B, NCH], F32, tag="pb", name="pb_sb")
            nc.sync.dma_start(out=pb_sb[:], in_=pbias[:])
            mk_sb = const.tile([128, 2, 128], BF16, tag="mk", name="mk_sb")
            nc.scalar.dma_start(out=mk_sb[:], in_=mtri[:])
            ones_sb = const.tile([128, 64], BF16, tag="ones", name="ones_sb")
            nc.vector.memset(ones_sb[:], 1.0)

            # ---- persistent per-block tiles ----
            qT = [persist.tile([128, TB], BF16, tag=f"qT{i}", name=f"qT{i}")
                  for i in range(NB)]
            kT = [persist.tile([128, TB], BF16, tag=f"kT{i}", name=f"kT{i}")
                  for i in range(NB)]
            V = [persist.tile([128, CPB, 128], BF16, tag=f"V{i}", name=f"V{i}")
                  for i in range(NB)]

            GRP = 4
            W = GRP * TB

            def qkv_group(g, split=4):
                xt = xp.tile([128, KC, W], BF16, tag="xt", name="xt")
                # split loads across the two HWDGE queues (sync + scalar)
                step = W // split
                for s in range(split):
                    for kc in range(KC):
                        eng = nc.sync if (kc + s) % 2 == 0 else nc.scalar
                        eng.dma_start(
                            out=xt[:, kc, s * step:(s + 1) * step],
                            in_=xb[kc * 128:(kc + 1) * 128,
                                   g * W + s * step:g * W + (s + 1) * step])
                for tl in range(GRP):
                    tb = g * GRP + tl
                    ts = slice(tl * TB, (tl + 1) * TB)
                    for fc in range(2):
                        ps = psmm.tile([128, TB], F32, tag="ps", name="ps")
                        for kc in range(KC):
                            nc.tensor.matmul(
                                ps[:], lhsT=wqk_sb[:, kc, fc, :],
                                rhs=xt[:, kc, ts],
                                start=(kc == 0), stop=(kc == KC - 1))
                        if fc == 0:
                            nc.vector.tensor_scalar_add(
                                out=qT[tb][:], in0=ps[:], scalar1=bq_sb[:])
                        else:
                            nc.vector.tensor_copy(kT[tb][:], ps[:])
                    # V^T then transpose to token-major fp8
                    ps = psmm.tile([128, TB], F32, tag="ps", name="ps")
                    for kc in range(KC):
                        nc.tensor.matmul(
                            ps[:], lhsT=wv_sb[:, kc, :],
                            rhs=xt[:, kc, ts],
                            start=(kc == 0), stop=(kc == KC - 1))
                    vs = vp.tile([128, TB], BF16, tag="vs", name="vs")
                    nc.vector.tensor_copy(vs[:], ps[:])
                    nc.sync.dma_start_transpose(out=V[tb][:], in_=vs[:])

            # ---- attention + inline projection ----
            def attn_block(b, qb, out_eng):
                gb = b * BPB + qb
                diag0 = qb * CPB
                nchunks = diag0 + CPB
                psO = psa.tile([128, TB], F32, tag="psa", name="psO")
                psD = psa.tile([128, TB], F32, tag="psa", name="psD")
                def s_exp(ci):
                    blk = b * BPB + ci // CPB
                    cl = ci % CPB
                    is_diag = ci >= diag0
                    o = (ci - diag0) * 128 if is_diag else 0
                    Wc = TB - o
                    psS = pss.tile([128, 2, TB], F32, tag="psS", name="psS")
                    for h in range(HPC):
                        nc.tensor.matmul(
                            psS[:, h, 0:Wc],
                            lhsT=kT[blk][h * 64:(h + 1) * 64,
                                         cl * 128:(cl + 1) * 128],
                            rhs=qT[gb][h * 64:(h + 1) * 64, o:TB],
                            start=True, stop=True)
                    pt = pp.tile([128, 2, TB], BF16, tag="pt", name="pt")
                    nc.scalar.activation(
                        out=pt[:, :, 0:Wc], in_=psS[:, :, 0:Wc],
                        func=AF.Exp, bias=pb_sb[:, b, ci:ci + 1], scale=scale)
                    if is_diag:
                        nc.vector.tensor_mul(
                            pt[:, :, 0:128], pt[:, :, 0:128], mk_sb[:])
                    return pt, o, Wc

                def pv_den(ci, pt, o, Wc):
                    blk = b * BPB + ci // CPB
                    cl = ci % CPB
                    start = (ci == 0)
                    last = (ci == nchunks - 1)
                    for h in range(HPC):
                        nc.tensor.matmul(
                            psO[h * 64:(h + 1) * 64, o:TB],
                            lhsT=V[blk][:, cl, h * 64:(h + 1) * 64],
                            rhs=pt[:, h, 0:Wc],
                            start=start, stop=last,
                            tile_position=(0, h * 64))
                    for h in range(HPC):
                        nc.tensor.matmul(
                            psD[h * 64:(h + 1) * 64, o:TB],
                            lhsT=ones_sb[:],
                            rhs=pt[:, h, 0:Wc],
                            start=start, stop=last,
                            tile_position=(0, h * 64))

                for c0 in range(0, nchunks, 2):
                    r0 = s_exp(c0)
                    r1 = s_exp(c0 + 1)
                    pv_den(c0, *r0)
                    pv_den(c0 + 1, *r1)
                # normalize: A^T = O^T * (1/denom)
                rt = rp.tile([128, TB], F32, tag="rt", name="rt")
                nc.vector.reciprocal_approx_fast(out=rt[:], in_=psD[:])
                at = pp.tile([128, TB], BF16, tag="at", name="at")
                nc.vector.tensor_mul(at[:], psO[:], rt[:])
                # inline output projection
                for fc in range(C // 128):
                    ps = psmm.tile([128, TB], F32, tag="ps", name="ps")
                    nc.tensor.matmul(ps[:],
                                     lhsT=wp_sb[:, fc * 128:(fc + 1) * 128],
                                     rhs=at[:], start=True, stop=True)
                    ot = op.tile([128, TB], BF16, tag="ot", name="ot")
                    nc.vector.tensor_copy(ot[:], ps[:])
                    out_eng.dma_start(
                        out=outT[fc * 128:(fc + 1) * 128,
                                 gb * TB:(gb + 1) * TB],
                        in_=ot[:])

            # emission: qkv group g covers batch g; weave attention between
            qkv_group(0, split=8)
            qkv_group(1)
            qkv_group(2)
            for qb in range(BPB):
                attn_block(0, qb, nc.sync)
            qkv_group(3)
            for qb in range(BPB):
                attn_block(1, qb, nc.scalar)
            for qb in range(BPB):
                attn_block(2, qb, nc.sync)
            for qb in reversed(range(BPB)):
                attn_block(3, qb, nc.scalar)

    nc.compile()
    return nc


def prep_core_inputs(x, key_padding_mask, W_qkv, b_qkv, W_proj,
                     n_cores=8, TB=512):
    """Host-side sharding: build the per-core input maps."""
    import numpy as np
    import ml_dtypes

    BFD = ml_dtypes.bfloat16
    B, T, C = x.shape
    D = 64
    H = C // D
    HPC = H // n_cores
    BT = B * T
    KC = C // 128
    NCH = T // 128

    xb = np.ascontiguousarray(x.reshape(BT, C).T).astype(BFD)       # [C, BT]

    pb = np.where(key_padding_mask, np.float32(-1e30),
                  np.float32(0.0)).astype(np.float32)
    pb = np.ascontiguousarray(
        pb.reshape(B, NCH, 128).transpose(2, 0, 1)).astype(np.float32)

    p = np.arange(128)[:, None]
    j = np.arange(128)[None, :]
    tri = (p <= j).astype(np.float32)                               # [128,128]
    mtri = np.repeat(tri[:, None, :], 2, axis=1).astype(ml_dtypes.bfloat16)

    in_maps = []
    for c in range(n_cores):
        hs = [HPC * c + i for i in range(HPC)]
        qcols = np.concatenate([h * D + np.arange(D) for h in hs])
        wq = W_qkv[:, qcols]                                        # [C,128]
        wk = W_qkv[:, C + qcols]
        wv = W_qkv[:, 2 * C + qcols]
        wqkc = np.stack([wq.reshape(KC, 128, 128),
                         wk.reshape(KC, 128, 128)], axis=2)         # [KC,128,2,128]
        wqkc = np.ascontiguousarray(wqkc.transpose(1, 0, 2, 3)).astype(BFD)
        wvc = np.ascontiguousarray(
            wv.reshape(KC, 128, 128).transpose(1, 0, 2)).astype(BFD)
        bqc = np.ascontiguousarray(
            b_qkv[qcols].reshape(128, 1)).astype(np.float32)
        wpc = np.ascontiguousarray(
            W_proj[qcols, :]).astype(ml_dtypes.bfloat16)
        in_maps.append({
            "xb": xb, "wqk": wqkc, "wv": wvc, "wp": wpc,
            "bq": bqc, "pbias": pb, "mtri": mtri,
        })
    return in_maps


def combine_outputs(results, B, T, C, b_qkv, b_proj, W_proj):
    import numpy as np
    acc = np.zeros((C, B * T), np.float32)
    for r in results:
        acc += np.asarray(r["outT"]).astype(np.float32)
    b_v = np.asarray(b_qkv, np.float32)[2 * C:3 * C]
    b_out = np.asarray(b_proj, np.float32) + \
        np.asarray(W_proj, np.float32).T @ b_v
    out = acc.T.reshape(B, T, C) + b_out
    return out.astype(np.float32)


# ---------------------------------------------------------------------------
# Self-contained entry point for the grading harness.
# ---------------------------------------------------------------------------
import numpy as np

_NC_CACHE = {}


def _get_nc():
    if "nc" not in _NC_CACHE:
        _NC_CACHE["nc"] = build_nc(B=4, T=2048, C=1024, num_devices=8)
    return _NC_CACHE["nc"]


def kernel(x, key_padding_mask, W_qkv, b_qkv, W_proj, b_proj):
    from concourse.bass_utils import run_bass_kernel_spmd

    x = np.asarray(x, dtype=np.float32)
    key_padding_mask = np.asarray(key_padding_mask).astype(bool)
    W_qkv = np.asarray(W_qkv, dtype=np.float32)
    b_qkv = np.asarray(b_qkv, dtype=np.float32)
    W_proj = np.asarray(W_proj, dtype=np.float32)
    b_proj = np.asarray(b_proj, dtype=np.float32)

    B, T, C = x.shape
    nc = _get_nc()
    in_maps = prep_core_inputs(x, key_padding_mask, W_qkv, b_qkv, W_proj,
                               n_cores=8)
    res = run_bass_kernel_spmd(nc, in_maps, list(range(8)))
    return combine_outputs(res.results, B, T, C, b_qkv, b_proj, W_proj)


# revision 13
# speedup vs baseline: 1.1976x; 1.1771x over previous
"""Bass/Tile kernel for causal self-attention, head-sharded across 8 cores.

Per-core layout (core c owns heads 2c, 2c+1):
  xb    [C, B*T]         bf16   x transposed (feature-major), same on all cores
  wqk   [128, KC, 2, 128] bf16  W_qkv q/k column-slices; fc0=q, fc1=k,
                                cols = [h0 d0-63 | h1 d0-63]
  wv    [128, KC, 128]   bf16   W_qkv v column-slice
  wp    [128, C]         bf16   W_proj row-slice (this core's 128 head dims)
  bq    [128, 1]         f32    q bias (k bias is absorbed: softmax is
                                invariant to per-query logit shifts, and
                                (q+bq)@k reproduces the only surviving term;
                                v bias commutes to host as W_proj^T @ b_v)
  pbias [128, B, T/128]  f32    per-key bias: 0 or -1e30
  mtri  [128, 2, 128]    bf16   lower-triangular causal mask
  outT  [C, B*T]         bf16   partial projection output (pre-bias)

Compute:
  1) QKV projection in bf16 -> qT (bias-added) / kT tiles;
     V^T -> DMA-transpose -> token-major V bf16.
  2) per (b, q-block): key chunks processed diagonal-first in descending
     offset, each clipped to its causal query range [o, 512); the newly
     opened query range gets start=True on the psO/psD accumulation.
     S^T bf16 per chunk, P = exp(scale*S + pbias) bf16 on ACT, triangular
     mask mul (first 128 cols only) on GpSimd, PV + denominator bf16 with
     two heads column-packed via tile_position.
  3) A = O * recip(denom); inline bf16 output projection; bf16 DMA out.
"""

import concourse.bass as bass
import concourse.mybir as mybir
import concourse.tile as tile
from concourse import bacc

F32 = mybir.dt.float32
BF16 = mybir.dt.bfloat16
F8 = mybir.dt.float8e4
AF = mybir.ActivationFunctionType
DR = mybir.MatmulPerfMode.DoubleRow

SHIFT = 0.0
_NOCLIP = False


def build_nc(B=4, T=2048, C=1024, HPC=2, D=64, TB=512, num_devices=8,
             scale=None):
    if scale is None:
        scale = D ** -0.5
    NT = B * T
    NB = NT // TB              # 512-token blocks (16)
    BPB = T // TB              # blocks per batch (4)
    CPB = TB // 128            # 128-key chunks per block (4)
    NCH = T // 128             # key chunks per batch (16)
    KC = C // 128              # contraction chunks (8)
    assert HPC == 2 and HPC * D == 128 and KC % 2 == 0 and CPB == 4

    nc = bacc.Bacc("TRN2", target_bir_lowering=False, debug=False,
                   num_devices=num_devices)

    xb = nc.dram_tensor("xb", [C, NT], BF16, kind="ExternalInput")
    wqk = nc.dram_tensor("wqk", [128, KC, 2, 128], BF16, kind="ExternalInput")
    wv = nc.dram_tensor("wv", [128, KC, 128], BF16, kind="ExternalInput")
    wp = nc.dram_tensor("wp", [128, C], BF16, kind="ExternalInput")
    bq = nc.dram_tensor("bq", [128, 1], F32, kind="ExternalInput")
    pbias = nc.dram_tensor("pbias", [128, B, NCH], F32, kind="ExternalInput")
    mtri = nc.dram_tensor("mtri", [128, 2, 128], BF16, kind="ExternalInput")
    outT = nc.dram_tensor("outT", [C, NT], BF16, kind="ExternalOutput")

    with tile.TileContext(nc) as tc:
        with (
            tc.tile_pool(name="const", bufs=1) as const,
            tc.tile_pool(name="persist", bufs=1) as persist,
            tc.tile_pool(name="xp", bufs=2) as xp,
            tc.tile_pool(name="vp", bufs=3) as vp,
            tc.tile_pool(name="pp", bufs=6) as pp,
            tc.tile_pool(name="rp", bufs=2) as rp,
            tc.tile_pool(name="op", bufs=10) as op,
            tc.tile_pool(name="psmm", bufs=2, space="PSUM") as psmm,
            tc.tile_pool(name="psa", bufs=2, space="PSUM") as psa,
            tc.tile_pool(name="pss", bufs=2, space="PSUM") as pss,
        ):
            # ---- constants ----
            wqk_sb = const.tile([128, KC, 2, 128], BF16, tag="wqk", name="wqk_sb")
            nc.sync.dma_start(out=wqk_sb[:], in_=wqk[:])
            wv_sb = const.tile([128, KC, 128], BF16, tag="wv", name="wv_sb")
            nc.sync.dma_start(out=wv_sb[:], in_=wv[:])
            wp_sb = const.tile([128, C], BF16, tag="wp", name="wp_sb")
            nc.scalar.dma_start(out=wp_sb[:], in_=wp[:])
            bq_sb = const.tile([128, 1], F32, tag="bq", name="bq_sb")
            nc.sync.dma_start(out=bq_sb[:], in_=bq[:])
            pb_sb = const.tile([128, B, NCH], F32, tag="pb", name="pb_sb")
            nc.sync.dma_start(out=pb_sb[:], in_=pbias[:])
            mk_sb = const.tile([128, 2, 128], BF16, tag="mk", name="mk_sb")
            nc.scalar.dma_start(out=mk_sb[:], in_=mtri[:])
            ones_sb = const.tile([128, 64], BF16, tag="ones", name="ones_sb")
            nc.vector.memset(ones_sb[:], 1.0)

            # ---- persistent per-block tiles ----
            qT = [persist.tile([128, TB], BF16, tag=f"qT{i}", name=f"qT{i}")
                  for i in range(NB)]
            kT = [persist.tile([128, TB], BF16, tag=f"kT{i}", name=f"kT{i}")
                  for i in range(NB)]
            V = [persist.tile([128, CPB, 128], BF16, tag=f"V{i}", name=f"V{i}")
                  for i in range(NB)]

            GRP = 4
            W = GRP * TB

            def qkv_group(g, split=4):
                xt = xp.tile([128, KC, W], BF16, tag="xt", name="xt")
                # split loads across the two HWDGE queues (sync + scalar)
                step = W // split
                for s in range(split):
                    for kc in range(KC):
                        eng = nc.sync if (kc + s) % 2 == 0 else nc.scalar
                        eng.dma_start(
                            out=xt[:, kc, s * step:(s + 1) * step],
                            in_=xb[kc * 128:(kc + 1) * 128,
                                   g * W + s * step:g * W + (s + 1) * step])
                for tl in range(GRP):
                    tb = g * GRP + tl
                    ts = slice(tl * TB, (tl + 1) * TB)
                    for fc in range(2):
                        ps = psmm.tile([128, TB], F32, tag="ps", name="ps")
                        for kc in range(KC):
                            nc.tensor.matmul(
                                ps[:], lhsT=wqk_sb[:, kc, fc, :],
                                rhs=xt[:, kc, ts],
                                start=(kc == 0), stop=(kc == KC - 1))
                        if fc == 0:
                            nc.vector.tensor_scalar_add(
                                out=qT[tb][:], in0=ps[:], scalar1=bq_sb[:])
                        else:
                            nc.vector.tensor_copy(kT[tb][:], ps[:])
                    # V^T then transpose to token-major fp8
                    ps = psmm.tile([128, TB], F32, tag="ps", name="ps")
                    for kc in range(KC):
                        nc.tensor.matmul(
                            ps[:], lhsT=wv_sb[:, kc, :],
                            rhs=xt[:, kc, ts],
                            start=(kc == 0), stop=(kc == KC - 1))
                    vs = vp.tile([128, TB], BF16, tag="vs", name="vs")
                    nc.vector.tensor_copy(vs[:], ps[:])
                    nc.sync.dma_start_transpose(out=V[tb][:], in_=vs[:])

            # ---- attention + inline projection ----
            def attn_block(b, qb, out_eng):
                gb = b * BPB + qb
                diag0 = qb * CPB
                nchunks = diag0 + CPB
                psO = psa.tile([128, TB], F32, tag="psa", name="psO")
                psD = psa.tile([128, TB], F32, tag="psa", name="psD")
                def s_exp(ci):
                    blk = b * BPB + ci // CPB
                    cl = ci % CPB
                    is_diag = ci >= diag0
                    o = (ci - diag0) * 128 if is_diag else 0
                    Wc = TB - o
                    psS = pss.tile([128, 2, TB], F32, tag="psS", name="psS")
                    for h in range(HPC):
                        nc.tensor.matmul(
                            psS[:, h, 0:Wc],
                            lhsT=kT[blk][h * 64:(h + 1) * 64,
                                         cl * 128:(cl + 1) * 128],
                            rhs=qT[gb][h * 64:(h + 1) * 64, o:TB],
                            start=True, stop=True)
                    pt = pp.tile([128, 2, TB], BF16, tag="pt", name="pt")
                    nc.scalar.activation(
                        out=pt[:, :, 0:Wc], in_=psS[:, :, 0:Wc],
                        func=AF.Exp, bias=pb_sb[:, b, ci:ci + 1], scale=scale)
                    if is_diag:
                        nc.vector.tensor_mul(
                            pt[:, :, 0:128], pt[:, :, 0:128], mk_sb[:])
                    return pt, o, Wc

                def pv_den(ci, pt, o, Wc):
                    blk = b * BPB + ci // CPB
                    cl = ci % CPB
                    start = (ci == 0)
                    last = (ci == nchunks - 1)
                    for h in range(HPC):
                        nc.tensor.matmul(
                            psO[h * 64:(h + 1) * 64, o:TB],
                            lhsT=V[blk][:, cl, h * 64:(h + 1) * 64],
                            rhs=pt[:, h, 0:Wc],
                            start=start, stop=last,
                            tile_position=(0, h * 64))
                    for h in range(HPC):
                        nc.tensor.matmul(
                            psD[h * 64:(h + 1) * 64, o:TB],
                            lhsT=ones_sb[:],
                            rhs=pt[:, h, 0:Wc],
                            start=start, stop=last,
                            tile_position=(0, h * 64))

                for c0 in range(0, nchunks, 2):
                    r0 = s_exp(c0)
                    r1 = s_exp(c0 + 1)
                    pv_den(c0, *r0)
                    pv_den(c0 + 1, *r1)
                # normalize: A^T = O^T * (1/denom)
                rt = rp.tile([128, TB], F32, tag="rt", name="rt")
                nc.vector.reciprocal_approx_fast(out=rt[:], in_=psD[:])
                at = pp.tile([128, TB], BF16, tag="at", name="at")
                nc.vector.tensor_mul(at[:], psO[:], rt[:])
                # inline output projection
                for fc in range(C // 128):
                    ps = psmm.tile([128, TB], F32, tag="ps", name="ps")
                    nc.tensor.matmul(ps[:],
                                     lhsT=wp_sb[:, fc * 128:(fc + 1) * 128],
                                     rhs=at[:], start=True, stop=True)
                    ot = op.tile([128, TB], BF16, tag="ot", name="ot")
                    nc.vector.tensor_copy(ot[:], ps[:])
                    out_eng.dma_start(
                        out=outT[fc * 128:(fc + 1) * 128,
                                 gb * TB:(gb + 1) * TB],
                        in_=ot[:])

            # emission: qkv group g covers batch g; weave attention between
            qkv_group(0, split=8)
            for qb in range(BPB):
                attn_block(0, qb, nc.gpsimd)
            qkv_group(1)
            for qb in range(BPB):
                attn_block(1, qb, nc.gpsimd)
            qkv_group(2)
            for qb in range(BPB):
                attn_block(2, qb, nc.gpsimd)
            qkv_group(3)
            for qb in reversed(range(BPB)):
                attn_block(3, qb, nc.gpsimd)

    nc.compile()
    return nc


def prep_core_inputs(x, key_padding_mask, W_qkv, b_qkv, W_proj,
                     n_cores=8, TB=512):
    """Host-side sharding: build the per-core input maps."""
    import numpy as np
    import ml_dtypes

    BFD = ml_dtypes.bfloat16
    B, T, C = x.shape
    D = 64
    H = C // D
    HPC = H // n_cores
    BT = B * T
    KC = C // 128
    NCH = T // 128

    xb = np.ascontiguousarray(x.reshape(BT, C).T).astype(BFD)       # [C, BT]

    pb = np.where(key_padding_mask, np.float32(-1e30),
                  np.float32(0.0)).astype(np.float32)
    pb = np.ascontiguousarray(
        pb.reshape(B, NCH, 128).transpose(2, 0, 1)).astype(np.float32)

    p = np.arange(128)[:, None]
    j = np.arange(128)[None, :]
    tri = (p <= j).astype(np.float32)                               # [128,128]
    mtri = np.repeat(tri[:, None, :], 2, axis=1).astype(ml_dtypes.bfloat16)

    in_maps = []
    for c in range(n_cores):
        hs = [HPC * c + i for i in range(HPC)]
        qcols = np.concatenate([h * D + np.arange(D) for h in hs])
        wq = W_qkv[:, qcols]                                        # [C,128]
        wk = W_qkv[:, C + qcols]
        wv = W_qkv[:, 2 * C + qcols]
        wqkc = np.stack([wq.reshape(KC, 128, 128),
                         wk.reshape(KC, 128, 128)], axis=2)         # [KC,128,2,128]
        wqkc = np.ascontiguousarray(wqkc.transpose(1, 0, 2, 3)).astype(BFD)
        wvc = np.ascontiguousarray(
            wv.reshape(KC, 128, 128).transpose(1, 0, 2)).astype(BFD)
        bqc = np.ascontiguousarray(
            b_qkv[qcols].reshape(128, 1)).astype(np.float32)
        wpc = np.ascontiguousarray(
            W_proj[qcols, :]).astype(ml_dtypes.bfloat16)
        in_maps.append({
            "xb": xb, "wqk": wqkc, "wv": wvc, "wp": wpc,
            "bq": bqc, "pbias": pb, "mtri": mtri,
        })
    return in_maps


def combine_outputs(results, B, T, C, b_qkv, b_proj, W_proj):
    import numpy as np
    acc = np.zeros((C, B * T), np.float32)
    for r in results:
        acc += np.asarray(r["outT"]).astype(np.float32)
    b_v = np.asarray(b_qkv, np.float32)[2 * C:3 * C]
    b_out = np.asarray(b_proj, np.float32) + \
        np.asarray(W_proj, np.float32).T @ b_v
    out = acc.T.reshape(B, T, C) + b_out
    return out.astype(np.float32)


# ---------------------------------------------------------------------------
# Self-contained entry point for the grading harness.
# ---------------------------------------------------------------------------
import numpy as np

_NC_CACHE = {}


def _get_nc():
    if "nc" not in _NC_CACHE:
        _NC_CACHE["nc"] = build_nc(B=4, T=2048, C=1024, num_devices=8)
    return _NC_CACHE["nc"]


def kernel(x, key_padding_mask, W_qkv, b_qkv, W_proj, b_proj):
    from concourse.bass_utils import run_bass_kernel_spmd

    x = np.asarray(x, dtype=np.float32)
    key_padding_mask = np.asarray(key_padding_mask).astype(bool)
    W_qkv = np.asarray(W_qkv, dtype=np.float32)
    b_qkv = np.asarray(b_qkv, dtype=np.float32)
    W_proj = np.asarray(W_proj, dtype=np.float32)
    b_proj = np.asarray(b_proj, dtype=np.float32)

    B, T, C = x.shape
    nc = _get_nc()
    in_maps = prep_core_inputs(x, key_padding_mask, W_qkv, b_qkv, W_proj,
                               n_cores=8)
    res = run_bass_kernel_spmd(nc, in_maps, list(range(8)))
    return combine_outputs(res.results, B, T, C, b_qkv, b_proj, W_proj)


# revision 14
# speedup vs baseline: 1.2355x; 1.0316x over previous
"""Bass/Tile kernel for causal self-attention, head-sharded across 8 cores.

Per-core layout (core c owns heads 2c, 2c+1):
  xb    [C, B*T]         bf16   x transposed (feature-major), same on all cores
  wqk   [128, KC, 2, 128] bf16  W_qkv q/k column-slices; fc0=q, fc1=k,
                                cols = [h0 d0-63 | h1 d0-63]
  wv    [128, KC, 128]   bf16   W_qkv v column-slice
  wp    [128, C]         bf16   W_proj row-slice (this core's 128 head dims)
  bq    [128, 1]         f32    q bias (k bias is absorbed: softmax is
                                invariant to per-query logit shifts, and
                                (q+bq)@k reproduces the only surviving term;
                                v bias commutes to host as W_proj^T @ b_v)
  pbias [128, B, T/128]  f32    per-key bias: 0 or -1e30
  mtri  [128, 2, 128]    bf16   lower-triangular causal mask
  outT  [C, B*T]         bf16   partial projection output (pre-bias)

Compute:
  1) QKV projection in bf16 -> qT (bias-added) / kT tiles;
     V^T -> DMA-transpose -> token-major V bf16.
  2) per (b, q-block): key chunks processed diagonal-first in descending
     offset, each clipped to its causal query range [o, 512); the newly
     opened query range gets start=True on the psO/psD accumulation.
     S^T bf16 per chunk, P = exp(scale*S + pbias) bf16 on ACT, triangular
     mask mul (first 128 cols only) on GpSimd, PV + denominator bf16 with
     two heads column-packed via tile_position.
  3) A = O * recip(denom); inline bf16 output projection; bf16 DMA out.
"""

import concourse.bass as bass
import concourse.mybir as mybir
import concourse.tile as tile
from concourse import bacc

F32 = mybir.dt.float32
BF16 = mybir.dt.bfloat16
F8 = mybir.dt.float8e4
AF = mybir.ActivationFunctionType
DR = mybir.MatmulPerfMode.DoubleRow

SHIFT = 0.0
_NOCLIP = False


def build_nc(B=4, T=2048, C=1024, HPC=2, D=64, TB=512, num_devices=8,
             scale=None):
    if scale is None:
        scale = D ** -0.5
    NT = B * T
    NB = NT // TB              # 512-token blocks (16)
    BPB = T // TB              # blocks per batch (4)
    CPB = TB // 128            # 128-key chunks per block (4)
    NCH = T // 128             # key chunks per batch (16)
    KC = C // 128              # contraction chunks (8)
    assert HPC == 2 and HPC * D == 128 and KC % 2 == 0 and CPB == 4

    nc = bacc.Bacc("TRN2", target_bir_lowering=False, debug=False,
                   num_devices=num_devices)

    xb = nc.dram_tensor("xb", [C, NT], BF16, kind="ExternalInput")
    wqk = nc.dram_tensor("wqk", [128, KC, 2, 128], BF16, kind="ExternalInput")
    wv = nc.dram_tensor("wv", [128, KC, 128], BF16, kind="ExternalInput")
    wp = nc.dram_tensor("wp", [128, C], BF16, kind="ExternalInput")
    bq = nc.dram_tensor("bq", [128, 1], F32, kind="ExternalInput")
    pbias = nc.dram_tensor("pbias", [128, B, NCH], F32, kind="ExternalInput")
    mtri = nc.dram_tensor("mtri", [128, 2, 128], BF16, kind="ExternalInput")
    outT = nc.dram_tensor("outT", [C, NT], BF16, kind="ExternalOutput")

    with tile.TileContext(nc) as tc:
        with (
            tc.tile_pool(name="const", bufs=1) as const,
            tc.tile_pool(name="persist", bufs=1) as persist,
            tc.tile_pool(name="xp", bufs=16) as xp,
            tc.tile_pool(name="vp", bufs=3) as vp,
            tc.tile_pool(name="pp", bufs=6) as pp,
            tc.tile_pool(name="rp", bufs=2) as rp,
            tc.tile_pool(name="op", bufs=10) as op,
            tc.tile_pool(name="psmm", bufs=2, space="PSUM") as psmm,
            tc.tile_pool(name="psa", bufs=2, space="PSUM") as psa,
            tc.tile_pool(name="pss", bufs=2, space="PSUM") as pss,
        ):
            # ---- constants ----
            wqk_sb = const.tile([128, KC, 2, 128], BF16, tag="wqk", name="wqk_sb")
            nc.sync.dma_start(out=wqk_sb[:], in_=wqk[:])
            wv_sb = const.tile([128, KC, 128], BF16, tag="wv", name="wv_sb")
            nc.sync.dma_start(out=wv_sb[:], in_=wv[:])
            wp_sb = const.tile([128, C], BF16, tag="wp", name="wp_sb")
            nc.scalar.dma_start(out=wp_sb[:], in_=wp[:])
            bq_sb = const.tile([128, 1], F32, tag="bq", name="bq_sb")
            nc.sync.dma_start(out=bq_sb[:], in_=bq[:])
            pb_sb = const.tile([128, B, NCH], F32, tag="pb", name="pb_sb")
            nc.sync.dma_start(out=pb_sb[:], in_=pbias[:])
            mk_sb = const.tile([128, 2, 128], BF16, tag="mk", name="mk_sb")
            nc.scalar.dma_start(out=mk_sb[:], in_=mtri[:])
            ones_sb = const.tile([128, 64], BF16, tag="ones", name="ones_sb")
            nc.vector.memset(ones_sb[:], 1.0)

            # ---- persistent per-block tiles ----
            qT = [persist.tile([128, TB], BF16, tag=f"qT{i}", name=f"qT{i}")
                  for i in range(NB)]
            kT = [persist.tile([128, TB], BF16, tag=f"kT{i}", name=f"kT{i}")
                  for i in range(NB)]
            V = [persist.tile([128, CPB, 128], BF16, tag=f"V{i}", name=f"V{i}")
                  for i in range(NB)]

            GRP = 4
            W = GRP * TB

            def qkv_group(g, split=2):
                x_tiles = []
                for kc in range(KC):
                    xt = xp.tile([128, W], BF16, tag="xt", name="xt")
                    step = W // split
                    for s in range(split):
                        eng = nc.sync if (kc + s) % 2 == 0 else nc.scalar
                        eng.dma_start(
                            out=xt[:, s * step:(s + 1) * step],
                            in_=xb[kc * 128:(kc + 1) * 128,
                                   g * W + s * step:g * W + (s + 1) * step])
                    x_tiles.append(xt)
                for tl in range(GRP):
                    tb = g * GRP + tl
                    ts = slice(tl * TB, (tl + 1) * TB)
                    for fc in range(2):
                        ps = psmm.tile([128, TB], F32, tag="ps", name="ps")
                        for kc in range(KC):
                            nc.tensor.matmul(
                                ps[:], lhsT=wqk_sb[:, kc, fc, :],
                                rhs=x_tiles[kc][:, ts],
                                start=(kc == 0), stop=(kc == KC - 1))
                        if fc == 0:
                            nc.vector.tensor_scalar_add(
                                out=qT[tb][:], in0=ps[:], scalar1=bq_sb[:])
                        else:
                            nc.vector.tensor_copy(kT[tb][:], ps[:])
                    # V^T then transpose to token-major V
                    ps = psmm.tile([128, TB], F32, tag="ps", name="ps")
                    for kc in range(KC):
                        nc.tensor.matmul(
                            ps[:], lhsT=wv_sb[:, kc, :],
                            rhs=x_tiles[kc][:, ts],
                            start=(kc == 0), stop=(kc == KC - 1))
                    vs = vp.tile([128, TB], BF16, tag="vs", name="vs")
                    nc.vector.tensor_copy(vs[:], ps[:])
                    nc.sync.dma_start_transpose(out=V[tb][:], in_=vs[:])

            # ---- attention + inline projection ----
            def attn_block(b, qb, out_eng):
                gb = b * BPB + qb
                diag0 = qb * CPB
                nchunks = diag0 + CPB
                psO = psa.tile([128, TB], F32, tag="psa", name="psO")
                psD = psa.tile([128, TB], F32, tag="psa", name="psD")
                def s_exp(ci):
                    blk = b * BPB + ci // CPB
                    cl = ci % CPB
                    is_diag = ci >= diag0
                    o = (ci - diag0) * 128 if is_diag else 0
                    Wc = TB - o
                    psS = pss.tile([128, 2, TB], F32, tag="psS", name="psS")
                    for h in range(HPC):
                        nc.tensor.matmul(
                            psS[:, h, 0:Wc],
                            lhsT=kT[blk][h * 64:(h + 1) * 64,
                                         cl * 128:(cl + 1) * 128],
                            rhs=qT[gb][h * 64:(h + 1) * 64, o:TB],
                            start=True, stop=True)
                    pt = pp.tile([128, 2, TB], BF16, tag="pt", name="pt")
                    nc.scalar.activation(
                        out=pt[:, :, 0:Wc], in_=psS[:, :, 0:Wc],
                        func=AF.Exp, bias=pb_sb[:, b, ci:ci + 1], scale=scale)
                    if is_diag:
                        nc.vector.tensor_mul(
                            pt[:, :, 0:128], pt[:, :, 0:128], mk_sb[:])
                    return pt, o, Wc

                def pv_den(ci, pt, o, Wc):
                    blk = b * BPB + ci // CPB
                    cl = ci % CPB
                    start = (ci == 0)
                    last = (ci == nchunks - 1)
                    for h in range(HPC):
                        nc.tensor.matmul(
                            psO[h * 64:(h + 1) * 64, o:TB],
                            lhsT=V[blk][:, cl, h * 64:(h + 1) * 64],
                            rhs=pt[:, h, 0:Wc],
                            start=start, stop=last,
                            tile_position=(0, h * 64))
                    for h in range(HPC):
                        nc.tensor.matmul(
                            psD[h * 64:(h + 1) * 64, o:TB],
                            lhsT=ones_sb[:],
                            rhs=pt[:, h, 0:Wc],
                            start=start, stop=last,
                            tile_position=(0, h * 64))

                for c0 in range(0, nchunks, 2):
                    r0 = s_exp(c0)
                    r1 = s_exp(c0 + 1)
                    pv_den(c0, *r0)
                    pv_den(c0 + 1, *r1)
                # normalize: A^T = O^T * (1/denom)
                rt = rp.tile([128, TB], F32, tag="rt", name="rt")
                nc.vector.reciprocal_approx_fast(out=rt[:], in_=psD[:])
                at = pp.tile([128, TB], BF16, tag="at", name="at")
                nc.vector.tensor_mul(at[:], psO[:], rt[:])
                # inline output projection
                for fc in range(C // 128):
                    ps = psmm.tile([128, TB], F32, tag="ps", name="ps")
                    nc.tensor.matmul(ps[:],
                                     lhsT=wp_sb[:, fc * 128:(fc + 1) * 128],
                                     rhs=at[:], start=True, stop=True)
                    ot = op.tile([128, TB], BF16, tag="ot", name="ot")
                    nc.vector.tensor_copy(ot[:], ps[:])
                    out_eng.dma_start(
                        out=outT[fc * 128:(fc + 1) * 128,
                                 gb * TB:(gb + 1) * TB],
                        in_=ot[:])

            # emission: qkv group g covers batch g; weave attention between
            qkv_group(0, split=4)
            for qb in range(BPB):
                attn_block(0, qb, nc.gpsimd)
            qkv_group(1)
            for qb in range(BPB):
                attn_block(1, qb, nc.gpsimd)
            qkv_group(2)
            for qb in range(BPB):
                attn_block(2, qb, nc.gpsimd)
            qkv_group(3)
            for qb in reversed(range(BPB)):
                attn_block(3, qb, nc.gpsimd)

    nc.compile()
    return nc


def prep_core_inputs(x, key_padding_mask, W_qkv, b_qkv, W_proj,
                     n_cores=8, TB=512):
    """Host-side sharding: build the per-core input maps."""
    import numpy as np
    import ml_dtypes

    BFD = ml_dtypes.bfloat16
    B, T, C = x.shape
    D = 64
    H = C // D
    HPC = H // n_cores
    BT = B * T
    KC = C // 128
    NCH = T // 128

    xb = np.ascontiguousarray(x.reshape(BT, C).T).astype(BFD)       # [C, BT]

    pb = np.where(key_padding_mask, np.float32(-1e30),
                  np.float32(0.0)).astype(np.float32)
    pb = np.ascontiguousarray(
        pb.reshape(B, NCH, 128).transpose(2, 0, 1)).astype(np.float32)

    p = np.arange(128)[:, None]
    j = np.arange(128)[None, :]
    tri = (p <= j).astype(np.float32)                               # [128,128]
    mtri = np.repeat(tri[:, None, :], 2, axis=1).astype(ml_dtypes.bfloat16)

    in_maps = []
    for c in range(n_cores):
        hs = [HPC * c + i for i in range(HPC)]
        qcols = np.concatenate([h * D + np.arange(D) for h in hs])
        wq = W_qkv[:, qcols]                                        # [C,128]
        wk = W_qkv[:, C + qcols]
        wv = W_qkv[:, 2 * C + qcols]
        wqkc = np.stack([wq.reshape(KC, 128, 128),
                         wk.reshape(KC, 128, 128)], axis=2)         # [KC,128,2,128]
        wqkc = np.ascontiguousarray(wqkc.transpose(1, 0, 2, 3)).astype(BFD)
        wvc = np.ascontiguousarray(
            wv.reshape(KC, 128, 128).transpose(1, 0, 2)).astype(BFD)
        bqc = np.ascontiguousarray(
            b_qkv[qcols].reshape(128, 1)).astype(np.float32)
        wpc = np.ascontiguousarray(
            W_proj[qcols, :]).astype(ml_dtypes.bfloat16)
        in_maps.append({
            "xb": xb, "wqk": wqkc, "wv": wvc, "wp": wpc,
            "bq": bqc, "pbias": pb, "mtri": mtri,
        })
    return in_maps


def combine_outputs(results, B, T, C, b_qkv, b_proj, W_proj):
    import numpy as np
    acc = np.zeros((C, B * T), np.float32)
    for r in results:
        acc += np.asarray(r["outT"]).astype(np.float32)
    b_v = np.asarray(b_qkv, np.float32)[2 * C:3 * C]
    b_out = np.asarray(b_proj, np.float32) + \
        np.asarray(W_proj, np.float32).T @ b_v
    out = acc.T.reshape(B, T, C) + b_out
    return out.astype(np.float32)


# ---------------------------------------------------------------------------
# Self-contained entry point for the grading harness.
# ---------------------------------------------------------------------------
import numpy as np

_NC_CACHE = {}


def _get_nc():
    if "nc" not in _NC_CACHE:
        _NC_CACHE["nc"] = build_nc(B=4, T=2048, C=1024, num_devices=8)
    return _NC_CACHE["nc"]


def kernel(x, key_padding_mask, W_qkv, b_qkv, W_proj, b_proj):
    from concourse.bass_utils import run_bass_kernel_spmd

    x = np.asarray(x, dtype=np.float32)
    key_padding_mask = np.asarray(key_padding_mask).astype(bool)
    W_qkv = np.asarray(W_qkv, dtype=np.float32)
    b_qkv = np.asarray(b_qkv, dtype=np.float32)
    W_proj = np.asarray(W_proj, dtype=np.float32)
    b_proj = np.asarray(b_proj, dtype=np.float32)

    B, T, C = x.shape
    nc = _get_nc()
    in_maps = prep_core_inputs(x, key_padding_mask, W_qkv, b_qkv, W_proj,
                               n_cores=8)
    res = run_bass_kernel_spmd(nc, in_maps, list(range(8)))
    return combine_outputs(res.results, B, T, C, b_qkv, b_proj, W_proj)


# revision 15
# speedup vs baseline: 1.2408x; 1.0043x over previous
"""Bass/Tile kernel for causal self-attention, head-sharded across 8 cores.

Per-core layout (core c owns heads 2c, 2c+1):
  xT    [C, B*T]        bf16   x transposed (feature-major), same on all cores
  wqkv  [128, KC, F]    bf16   W_qkv column-slice, f = [q_h0|q_h1|k_h0|k_h1|v_h0|v_h1]*64
  wproj [128, C]        bf16   W_proj row-slice (this core's 128 head dims)
  bqkv  [128, FC]       f32    b_qkv slice, partition-major per f-chunk
  pbias [128, B, T/128] f32    key-padding bias (0 or -1e30) per key
  masks [128, 2, 128]   bf16   lower-triangular causal mask (replicated per head)
  outT  [C, B*T]        bf16   partial projection output (pre-bias)

Phases (Tile schedules by dependency; emission order = per-engine queue order):
  1) qkvT = W_c^T x (+bias on DVE) -> qT/kT tiles bf16; V via DMA-transpose.
  2) per (b, q-block): per 128-key chunk: S^T both heads (clipped to the
     causal query range [o, 512) for diagonal chunks), P = exp(scale*S^T +
     pbias) on ACT, leading-128-column triangle mask multiply on DVE,
     PV + denominator matmuls (two heads column-packed, psO/psD accumulate
     over chunks; chunk 0 is always full-width so it opens the banks).
  3) A^T = O^T * recip(denom); inline bf16 output projection; bf16 DMA out.
"""

import concourse.bass as bass
import concourse.mybir as mybir
import concourse.tile as tile
from concourse import bacc

F32 = mybir.dt.float32
BF16 = mybir.dt.bfloat16
AF = mybir.ActivationFunctionType


def build_nc(B=4, T=2048, C=1024, HPC=2, D=64, TB=512, num_devices=8,
             scale=None, pad_bias=True):
    if scale is None:
        scale = D ** -0.5
    NT = B * T                 # total tokens
    NB = NT // TB              # 512-token blocks (global)
    BPB = T // TB              # blocks per batch
    CPB = TB // 128            # 128-chunks per block (4)
    NCH = T // 128             # key chunks per batch
    KC = C // 128              # contraction chunks for qkv matmul
    F = HPC * 3 * D            # per-core qkv features (384)
    FC = F // 128              # f-chunks (3)
    assert HPC == 2 and HPC * D == 128 and F % 128 == 0 and TB % 128 == 0

    nc = bacc.Bacc("TRN2", target_bir_lowering=False, debug=False,
                   num_devices=num_devices)

    xT = nc.dram_tensor("xT", [C, NT], BF16, kind="ExternalInput")
    wqkv = nc.dram_tensor("wqkv", [128, KC, F], BF16, kind="ExternalInput")
    wproj = nc.dram_tensor("wproj", [128, C], BF16, kind="ExternalInput")
    bqkv = nc.dram_tensor("bqkv", [128, FC], F32, kind="ExternalInput")
    pbias = nc.dram_tensor("pbias", [128, B, NCH], F32, kind="ExternalInput")
    masks = nc.dram_tensor("masks", [128, 2, 128], BF16, kind="ExternalInput")
    outT = nc.dram_tensor("outT", [C, NT], BF16, kind="ExternalOutput")

    with tile.TileContext(nc) as tc:
        with (
            tc.tile_pool(name="const", bufs=1) as const,
            tc.tile_pool(name="persist", bufs=1) as persist,
            tc.tile_pool(name="xp", bufs=16) as xp,
            tc.tile_pool(name="pp", bufs=10) as pp,
            tc.tile_pool(name="rp", bufs=2) as rp,
            tc.tile_pool(name="op", bufs=10) as op,
            tc.tile_pool(name="psmm", bufs=4, space="PSUM") as psmm,
            tc.tile_pool(name="pss", bufs=2, space="PSUM") as pss,
        ):
            # ---- constants ----
            w_sb = const.tile([128, KC, F], BF16, tag="w", name="w_sb")
            nc.gpsimd.dma_start(out=w_sb[:], in_=wqkv[:])
            wp_sb = const.tile([128, C], BF16, tag="wp", name="wp_sb")
            nc.gpsimd.dma_start(out=wp_sb[:], in_=wproj[:])
            bq_sb = const.tile([128, FC], F32, tag="bq", name="bq_sb")
            nc.gpsimd.dma_start(out=bq_sb[:], in_=bqkv[:])
            pb_sb = const.tile([128, B, NCH], F32, tag="pb", name="pb_sb")
            nc.gpsimd.dma_start(out=pb_sb[:], in_=pbias[:])
            # lower-triangular mask, replicated per head: [128, 2, 128]
            mk_sb = const.tile([128, 2, 128], BF16, tag="mk", name="mk_sb")
            nc.gpsimd.dma_start(out=mk_sb[:], in_=masks[:])
            ones_sb = const.tile([128, 64], BF16, tag="ones", name="ones_sb")
            nc.vector.memset(ones_sb[:], 1.0)

            # ---- persistent per-block tiles ----
            qT = [persist.tile([128, TB], BF16, tag=f"qT{i}", name=f"qT{i}")
                  for i in range(NB)]
            kT = [persist.tile([128, TB], BF16, tag=f"kT{i}", name=f"kT{i}")
                  for i in range(NB)]
            V = [persist.tile([128, CPB, 128], BF16, tag=f"V{i}", name=f"V{i}")
                 for i in range(NB)]

            # ---- phase 1: QKV projection ----
            GRP = 4
            assert NB % GRP == 0

            def qkv_group(g):
                x_tiles = []
                W = GRP * TB
                for kc in range(KC):
                    xt = xp.tile([128, W], BF16, tag="xt", name="xt")
                    nc.gpsimd.dma_start(
                        out=xt[:, 0:W // 2],
                        in_=xT[kc * 128:(kc + 1) * 128,
                               g * W:g * W + W // 2])
                    nc.sync.dma_start(
                        out=xt[:, W // 2:W],
                        in_=xT[kc * 128:(kc + 1) * 128,
                               g * W + W // 2:(g + 1) * W])
                    x_tiles.append(xt)
                for tl in range(GRP):
                    tb = g * GRP + tl
                    for fc in range(FC):
                        ps = psmm.tile([128, TB], F32, tag="ps", name="ps")
                        for kc in range(KC):
                            nc.tensor.matmul(
                                ps[:], lhsT=w_sb[:, kc, fc * 128:(fc + 1) * 128],
                                rhs=x_tiles[kc][:, tl * TB:(tl + 1) * TB],
                                start=(kc == 0), stop=(kc == KC - 1))
                        if fc == 0:
                            dest = qT[tb]
                        elif fc == 1:
                            dest = kT[tb]
                        else:
                            dest = persist.tile([128, TB], BF16,
                                                tag=f"vs{tb % 2}", name="vs")
                        nc.vector.tensor_scalar_add(
                            out=dest[:], in0=ps[:], scalar1=bq_sb[:, fc:fc + 1])
                        if fc == 2:
                            nc.sync.dma_start_transpose(
                                out=V[tb][:], in_=dest[:])

            # ---- phase 2+3: attention + inline projection ----
            def attn_block(b, qb):
                gb = b * BPB + qb
                nchunks = (qb + 1) * CPB
                psO = psmm.tile([128, TB], F32, tag="ps", name="psO")
                psD = psmm.tile([128, TB], F32, tag="ps", name="psD")
                assert nchunks % 2 == 0
                for c0 in range(0, nchunks, 2):
                    pts = []
                    for ci in (c0, c0 + 1):
                        cb = ci // CPB      # kT block within batch
                        cl = ci % CPB       # 128-chunk within that block
                        is_diag = ci >= qb * CPB
                        o = (ci - qb * CPB) * 128 if is_diag else 0
                        Wc = TB - o
                        ktile = kT[b * BPB + cb]
                        psS = pss.tile([128, 2 * TB], F32, tag="pss", name="psS")
                        for h in range(HPC):
                            nc.tensor.matmul(
                                psS[:, h * TB:h * TB + Wc],
                                lhsT=ktile[h * 64:(h + 1) * 64,
                                           cl * 128:(cl + 1) * 128],
                                rhs=qT[gb][h * 64:(h + 1) * 64, o:TB],
                                start=True, stop=True)
                        pt = pp.tile([128, 2, TB], BF16, tag="pt", name="pt")
                        nc.scalar.activation(
                            out=pt[:, :, 0:Wc],
                            in_=psS.rearrange("p (h t) -> p h t", h=2)[:, :, 0:Wc],
                            func=AF.Exp,
                            bias=pb_sb[:, b, ci:ci + 1], scale=scale)
                        if is_diag:  # leading 128-col triangle only
                            nc.vector.tensor_mul(
                                pt[:, :, 0:128], pt[:, :, 0:128], mk_sb[:])
                        pts.append((pt, o, Wc))
                    for j, ci in enumerate((c0, c0 + 1)):
                        vtile = V[b * BPB + ci // CPB]
                        pt, o, Wc = pts[j]
                        for h in range(HPC):
                            nc.tensor.matmul(
                                psO[h * 64:(h + 1) * 64, o:TB],
                                lhsT=vtile[:, ci % CPB, h * 64:(h + 1) * 64],
                                rhs=pt[:, h, 0:Wc],
                                start=(ci == 0), stop=(ci == nchunks - 1),
                                tile_position=(0, h * 64))
                    for j, ci in enumerate((c0, c0 + 1)):
                        pt, o, Wc = pts[j]
                        for h in range(HPC):
                            nc.tensor.matmul(
                                psD[h * 64:(h + 1) * 64, o:TB],
                                lhsT=ones_sb[:],
                                rhs=pt[:, h, 0:Wc],
                                start=(ci == 0), stop=(ci == nchunks - 1),
                                tile_position=(0, h * 64))
                # normalize: A^T = O^T * (1/denom)
                rt = rp.tile([128, TB], F32, tag="rt", name="rt")
                nc.vector.reciprocal_approx_fast(out=rt[:], in_=psD[:])
                at = pp.tile([128, TB], BF16, tag="at", name="at")
                nc.vector.tensor_mul(at[:], psO[:], rt[:])
                for fc in range(C // 128):
                    ps = psmm.tile([128, TB], F32, tag="ps", name="ps")
                    nc.tensor.matmul(ps[:],
                                     lhsT=wp_sb[:, fc * 128:(fc + 1) * 128],
                                     rhs=at[:], start=True, stop=True)
                    ot = op.tile([128, TB], BF16, tag="ot", name="ot")
                    nc.vector.tensor_copy(ot[:], ps[:])
                    nc.gpsimd.dma_start(
                        out=outT[fc * 128:(fc + 1) * 128,
                                 gb * TB:(gb + 1) * TB],
                        in_=ot[:])

            ngroups = NB // GRP
            emitted = 0
            qkv_group(0)
            for g in range(1, ngroups):
                qkv_group(g)
                if g >= 2:
                    b = emitted
                    for qb in range(BPB):
                        attn_block(b, qb)
                    emitted += 1
            for b in range(emitted, B):
                for qb in range(BPB):
                    attn_block(b, qb)

    nc.compile()
    return nc


def prep_core_inputs(x, key_padding_mask, W_qkv, b_qkv, W_proj,
                     n_cores=8, TB=512):
    import numpy as np
    import ml_dtypes

    B, T, C = x.shape
    D = 64
    H = C // D
    HPC = H // n_cores
    BT = B * T
    CPB = TB // 128

    xT = np.ascontiguousarray(
        x.reshape(BT, C).T).astype(ml_dtypes.bfloat16)          # [C, BT]

    pb = np.where(key_padding_mask, np.float32(-1e30),
                  np.float32(0.0)).astype(np.float32)           # [B, T]
    pb = np.ascontiguousarray(pb.reshape(B, T // 128, 128).transpose(2, 0, 1))

    p = np.arange(128)[:, None]
    j = np.arange(128)[None, :]
    mk = np.repeat((p <= j)[:, None, :], 2, axis=1)
    mk = mk.astype(ml_dtypes.bfloat16)                          # [128, 2, 128]

    KC = C // 128
    in_maps = []
    for c in range(n_cores):
        hs = [HPC * c + i for i in range(HPC)]
        cols = np.concatenate([
            np.concatenate([which * H * D + h * D + np.arange(D) for h in hs])
            for which in range(3)])                             # [F]
        Wc = W_qkv[:, cols]                                     # [C, F]
        F = Wc.shape[1]
        wq = np.ascontiguousarray(
            Wc.reshape(KC, 128, F).transpose(1, 0, 2)).astype(ml_dtypes.bfloat16)
        bq = np.ascontiguousarray(
            b_qkv[cols].reshape(F // 128, 128).T).astype(np.float32)
        rows = np.concatenate([h * D + np.arange(D) for h in hs])
        wp = np.ascontiguousarray(W_proj[rows, :]).astype(ml_dtypes.bfloat16)
        in_maps.append({
            "xT": xT, "wqkv": wq.reshape(128, KC, F), "wproj": wp,
            "bqkv": bq, "pbias": pb, "masks": mk,
        })
    return in_maps


def combine_outputs(results, B, T, C, b_proj):
    import numpy as np
    acc = np.asarray(results[0]["outT"]).astype(np.float32)
    for r in results[1:]:
        acc = acc + np.asarray(r["outT"]).astype(np.float32)
    out = acc.T.reshape(B, T, C) + b_proj.astype(np.float32)
    return out.astype(np.float32)


# ---------------------------------------------------------------------------
# Self-contained entry point for the grading harness.
# kernel(**inputs) takes the FULL unsharded inputs and returns the FULL
# output. Sharding: tensor-parallel over heads (2 heads per core, 8 cores);
# each core computes its QKV column-slice, attention for its heads, and a
# partial output projection (bf16); partials are summed on the host.
# ---------------------------------------------------------------------------
import numpy as np

_NC_CACHE = {}


def _get_nc():
    if "nc" not in _NC_CACHE:
        _NC_CACHE["nc"] = build_nc(B=4, T=2048, C=1024, num_devices=8)
    return _NC_CACHE["nc"]


def kernel(x, key_padding_mask, W_qkv, b_qkv, W_proj, b_proj):
    from concourse.bass_utils import run_bass_kernel_spmd

    x = np.asarray(x, dtype=np.float32)
    key_padding_mask = np.asarray(key_padding_mask).astype(bool)
    W_qkv = np.asarray(W_qkv, dtype=np.float32)
    b_qkv = np.asarray(b_qkv, dtype=np.float32)
    W_proj = np.asarray(W_proj, dtype=np.float32)
    b_proj = np.asarray(b_proj, dtype=np.float32)

    B, T, C = x.shape
    nc = _get_nc()
    in_maps = prep_core_inputs(x, key_padding_mask, W_qkv, b_qkv, W_proj,
                               n_cores=8)
    res = run_bass_kernel_spmd(nc, in_maps, list(range(8)))
    return combine_outputs(res.results, B, T, C, b_proj)


# revision 16
# speedup vs baseline: 1.2455x; 1.0038x over previous
"""Bass/Tile kernel for causal self-attention, head-sharded across 8 cores.

Per-core layout (core c owns heads 2c, 2c+1):
  xT    [C, B*T]        bf16   x transposed (feature-major), same on all cores
  wqkv  [128, KC, F]    bf16   W_qkv column-slice, f = [q_h0|q_h1|k_h0|k_h1|v_h0|v_h1]*64
  wproj [128, C]        bf16   W_proj row-slice (this core's 128 head dims)
  bqkv  [128, FC]       f32    b_qkv slice, partition-major per f-chunk
  pbias [128, B, T/128] f32    key-padding bias (0 or -1e30) per key
  masks [128, 2, 128]   bf16   lower-triangular causal mask (replicated per head)
  outT  [C, B*T]        bf16   partial projection output (pre-bias)

Phases (Tile schedules by dependency; emission order = per-engine queue order):
  1) qkvT = W_c^T x (+bias on DVE) -> qT/kT tiles bf16; V via DMA-transpose.
  2) per (b, q-block): per 128-key chunk: S^T both heads (clipped to the
     causal query range [o, 512) for diagonal chunks), P = exp(scale*S^T +
     pbias) on ACT, leading-128-column triangle mask multiply on DVE,
     PV + denominator matmuls (two heads column-packed, psO/psD accumulate
     over chunks; chunk 0 is always full-width so it opens the banks).
  3) A^T = O^T * recip(denom); inline bf16 output projection; bf16 DMA out.
"""

import concourse.bass as bass
import concourse.mybir as mybir
import concourse.tile as tile
from concourse import bacc

F32 = mybir.dt.float32
BF16 = mybir.dt.bfloat16
AF = mybir.ActivationFunctionType


def build_nc(B=4, T=2048, C=1024, HPC=2, D=64, TB=512, num_devices=8,
             scale=None, pad_bias=True):
    if scale is None:
        scale = D ** -0.5
    NT = B * T                 # total tokens
    NB = NT // TB              # 512-token blocks (global)
    BPB = T // TB              # blocks per batch
    CPB = TB // 128            # 128-chunks per block (4)
    NCH = T // 128             # key chunks per batch
    KC = C // 128              # contraction chunks for qkv matmul
    F = HPC * 3 * D            # per-core qkv features (384)
    FC = F // 128              # f-chunks (3)
    assert HPC == 2 and HPC * D == 128 and F % 128 == 0 and TB % 128 == 0

    nc = bacc.Bacc("TRN2", target_bir_lowering=False, debug=False,
                   num_devices=num_devices)

    xT = nc.dram_tensor("xT", [C, NT], BF16, kind="ExternalInput")
    wqkv = nc.dram_tensor("wqkv", [128, KC, F], BF16, kind="ExternalInput")
    wproj = nc.dram_tensor("wproj", [128, C], BF16, kind="ExternalInput")
    bqkv = nc.dram_tensor("bqkv", [128, FC], F32, kind="ExternalInput")
    pbias = nc.dram_tensor("pbias", [128, B, NCH], F32, kind="ExternalInput")
    masks = nc.dram_tensor("masks", [128, 2, 128], BF16, kind="ExternalInput")
    outT = nc.dram_tensor("outT", [C, NT], BF16, kind="ExternalOutput")

    with tile.TileContext(nc) as tc:
        with (
            tc.tile_pool(name="const", bufs=1) as const,
            tc.tile_pool(name="persist", bufs=1) as persist,
            tc.tile_pool(name="xp", bufs=16) as xp,
            tc.tile_pool(name="pp", bufs=10) as pp,
            tc.tile_pool(name="rp", bufs=2) as rp,
            tc.tile_pool(name="op", bufs=10) as op,
            tc.tile_pool(name="psmm", bufs=4, space="PSUM") as psmm,
            tc.tile_pool(name="pss", bufs=2, space="PSUM") as pss,
        ):
            # ---- constants ----
            w_sb = const.tile([128, KC, F], BF16, tag="w", name="w_sb")
            nc.gpsimd.dma_start(out=w_sb[:], in_=wqkv[:])
            wp_sb = const.tile([128, C], BF16, tag="wp", name="wp_sb")
            nc.gpsimd.dma_start(out=wp_sb[:], in_=wproj[:])
            bq_sb = const.tile([128, FC], F32, tag="bq", name="bq_sb")
            nc.gpsimd.dma_start(out=bq_sb[:], in_=bqkv[:])
            pb_sb = const.tile([128, B, NCH], F32, tag="pb", name="pb_sb")
            nc.gpsimd.dma_start(out=pb_sb[:], in_=pbias[:])
            # lower-triangular mask, replicated per head: [128, 2, 128]
            mk_sb = const.tile([128, 2, 128], BF16, tag="mk", name="mk_sb")
            nc.gpsimd.dma_start(out=mk_sb[:], in_=masks[:])
            ones_sb = const.tile([128, 64], BF16, tag="ones", name="ones_sb")
            nc.vector.memset(ones_sb[:], 1.0)

            # ---- persistent per-block tiles ----
            qT = [persist.tile([128, TB], BF16, tag=f"qT{i}", name=f"qT{i}")
                  for i in range(NB)]
            kT = [persist.tile([128, TB], BF16, tag=f"kT{i}", name=f"kT{i}")
                  for i in range(NB)]
            V = [persist.tile([128, CPB, 128], BF16, tag=f"V{i}", name=f"V{i}")
                 for i in range(NB)]

            # ---- phase 1: QKV projection ----
            GRP = 4
            assert NB % GRP == 0

            def qkv_group(g, t0=0, nt=GRP):
                x_tiles = []
                W = GRP * TB
                Wn = nt * TB
                base = g * W + t0 * TB
                for kc in range(KC):
                    xt = xp.tile([128, W], BF16, tag="xt", name="xt")
                    nc.gpsimd.dma_start(
                        out=xt[:, 0:Wn // 2],
                        in_=xT[kc * 128:(kc + 1) * 128,
                               base:base + Wn // 2])
                    nc.sync.dma_start(
                        out=xt[:, Wn // 2:Wn],
                        in_=xT[kc * 128:(kc + 1) * 128,
                               base + Wn // 2:base + Wn])
                    x_tiles.append(xt)
                for tl in range(nt):
                    tb = g * GRP + t0 + tl
                    for fc in range(FC):
                        ps = psmm.tile([128, TB], F32, tag="ps", name="ps")
                        for kc in range(KC):
                            nc.tensor.matmul(
                                ps[:], lhsT=w_sb[:, kc, fc * 128:(fc + 1) * 128],
                                rhs=x_tiles[kc][:, tl * TB:(tl + 1) * TB],
                                start=(kc == 0), stop=(kc == KC - 1))
                        if fc == 0:
                            dest = qT[tb]
                        elif fc == 1:
                            dest = kT[tb]
                        else:
                            dest = persist.tile([128, TB], BF16,
                                                tag=f"vs{tb % 2}", name="vs")
                        nc.vector.tensor_scalar_add(
                            out=dest[:], in0=ps[:], scalar1=bq_sb[:, fc:fc + 1])
                        if fc == 2:
                            nc.scalar.dma_start_transpose(
                                out=V[tb][:], in_=dest[:])

            # ---- phase 2+3: attention + inline projection ----
            def attn_block(b, qb):
                gb = b * BPB + qb
                nchunks = (qb + 1) * CPB
                psO = psmm.tile([128, TB], F32, tag="ps", name="psO")
                psD = psmm.tile([128, TB], F32, tag="ps", name="psD")
                assert nchunks % 2 == 0
                for c0 in range(0, nchunks, 2):
                    pts = []
                    for ci in (c0, c0 + 1):
                        cb = ci // CPB      # kT block within batch
                        cl = ci % CPB       # 128-chunk within that block
                        is_diag = ci >= qb * CPB
                        o = (ci - qb * CPB) * 128 if is_diag else 0
                        Wc = TB - o
                        ktile = kT[b * BPB + cb]
                        psS = pss.tile([128, 2 * TB], F32, tag="pss", name="psS")
                        for h in range(HPC):
                            nc.tensor.matmul(
                                psS[:, h * TB:h * TB + Wc],
                                lhsT=ktile[h * 64:(h + 1) * 64,
                                           cl * 128:(cl + 1) * 128],
                                rhs=qT[gb][h * 64:(h + 1) * 64, o:TB],
                                start=True, stop=True)
                        pt = pp.tile([128, 2, TB], BF16, tag="pt", name="pt")
                        nc.scalar.activation(
                            out=pt[:, :, 0:Wc],
                            in_=psS.rearrange("p (h t) -> p h t", h=2)[:, :, 0:Wc],
                            func=AF.Exp,
                            bias=pb_sb[:, b, ci:ci + 1], scale=scale)
                        if is_diag:  # leading 128-col triangle only
                            nc.vector.tensor_mul(
                                pt[:, :, 0:128], pt[:, :, 0:128], mk_sb[:])
                        pts.append((pt, o, Wc))
                    for j, ci in enumerate((c0, c0 + 1)):
                        vtile = V[b * BPB + ci // CPB]
                        pt, o, Wc = pts[j]
                        for h in range(HPC):
                            nc.tensor.matmul(
                                psO[h * 64:(h + 1) * 64, o:TB],
                                lhsT=vtile[:, ci % CPB, h * 64:(h + 1) * 64],
                                rhs=pt[:, h, 0:Wc],
                                start=(ci == 0), stop=(ci == nchunks - 1),
                                tile_position=(0, h * 64))
                    for j, ci in enumerate((c0, c0 + 1)):
                        pt, o, Wc = pts[j]
                        for h in range(HPC):
                            nc.tensor.matmul(
                                psD[h * 64:(h + 1) * 64, o:TB],
                                lhsT=ones_sb[:],
                                rhs=pt[:, h, 0:Wc],
                                start=(ci == 0), stop=(ci == nchunks - 1),
                                tile_position=(0, h * 64))
                # normalize: A^T = O^T * (1/denom)
                rt = rp.tile([128, TB], F32, tag="rt", name="rt")
                nc.vector.reciprocal_approx_fast(out=rt[:], in_=psD[:])
                at = pp.tile([128, TB], BF16, tag="at", name="at")
                nc.vector.tensor_mul(at[:], psO[:], rt[:])
                for fc in range(C // 128):
                    ps = psmm.tile([128, TB], F32, tag="ps", name="ps")
                    nc.tensor.matmul(ps[:],
                                     lhsT=wp_sb[:, fc * 128:(fc + 1) * 128],
                                     rhs=at[:], start=True, stop=True)
                    ot = op.tile([128, TB], BF16, tag="ot", name="ot")
                    nc.vector.tensor_copy(ot[:], ps[:])
                    nc.gpsimd.dma_start(
                        out=outT[fc * 128:(fc + 1) * 128,
                                 gb * TB:(gb + 1) * TB],
                        in_=ot[:])

            emitted = 0
            qkv_group(0, 0, 1)
            qkv_group(0, 1, 3)
            for g in range(1, NB // GRP):
                qkv_group(g)
                if g >= 2:
                    for qb in range(BPB):
                        attn_block(emitted, qb)
                    emitted += 1
            for b in range(emitted, B):
                for qb in range(BPB):
                    attn_block(b, qb)

    nc.compile()
    return nc


def prep_core_inputs(x, key_padding_mask, W_qkv, b_qkv, W_proj,
                     n_cores=8, TB=512):
    import numpy as np
    import ml_dtypes

    B, T, C = x.shape
    D = 64
    H = C // D
    HPC = H // n_cores
    BT = B * T
    CPB = TB // 128

    xT = np.ascontiguousarray(
        x.reshape(BT, C).T).astype(ml_dtypes.bfloat16)          # [C, BT]

    pb = np.where(key_padding_mask, np.float32(-1e30),
                  np.float32(0.0)).astype(np.float32)           # [B, T]
    pb = np.ascontiguousarray(pb.reshape(B, T // 128, 128).transpose(2, 0, 1))

    p = np.arange(128)[:, None]
    j = np.arange(128)[None, :]
    mk = np.repeat((p <= j)[:, None, :], 2, axis=1)
    mk = mk.astype(ml_dtypes.bfloat16)                          # [128, 2, 128]

    KC = C // 128
    in_maps = []
    for c in range(n_cores):
        hs = [HPC * c + i for i in range(HPC)]
        cols = np.concatenate([
            np.concatenate([which * H * D + h * D + np.arange(D) for h in hs])
            for which in range(3)])                             # [F]
        Wc = W_qkv[:, cols]                                     # [C, F]
        F = Wc.shape[1]
        wq = np.ascontiguousarray(
            Wc.reshape(KC, 128, F).transpose(1, 0, 2)).astype(ml_dtypes.bfloat16)
        bq = np.ascontiguousarray(
            b_qkv[cols].reshape(F // 128, 128).T).astype(np.float32)
        rows = np.concatenate([h * D + np.arange(D) for h in hs])
        wp = np.ascontiguousarray(W_proj[rows, :]).astype(ml_dtypes.bfloat16)
        in_maps.append({
            "xT": xT, "wqkv": wq.reshape(128, KC, F), "wproj": wp,
            "bqkv": bq, "pbias": pb, "masks": mk,
        })
    return in_maps


def combine_outputs(results, B, T, C, b_proj):
    import numpy as np
    acc = np.asarray(results[0]["outT"]).astype(np.float32)
    for r in results[1:]:
        acc = acc + np.asarray(r["outT"]).astype(np.float32)
    out = acc.T.reshape(B, T, C) + b_proj.astype(np.float32)
    return out.astype(np.float32)


# ---------------------------------------------------------------------------
# Self-contained entry point for the grading harness.
# kernel(**inputs) takes the FULL unsharded inputs and returns the FULL
# output. Sharding: tensor-parallel over heads (2 heads per core, 8 cores);
# each core computes its QKV column-slice, attention for its heads, and a
# partial output projection (bf16); partials are summed on the host.
# ---------------------------------------------------------------------------
import numpy as np

_NC_CACHE = {}


def _get_nc():
    if "nc" not in _NC_CACHE:
        _NC_CACHE["nc"] = build_nc(B=4, T=2048, C=1024, num_devices=8)
    return _NC_CACHE["nc"]


def kernel(x, key_padding_mask, W_qkv, b_qkv, W_proj, b_proj):
    from concourse.bass_utils import run_bass_kernel_spmd

    x = np.asarray(x, dtype=np.float32)
    key_padding_mask = np.asarray(key_padding_mask).astype(bool)
    W_qkv = np.asarray(W_qkv, dtype=np.float32)
    b_qkv = np.asarray(b_qkv, dtype=np.float32)
    W_proj = np.asarray(W_proj, dtype=np.float32)
    b_proj = np.asarray(b_proj, dtype=np.float32)

    B, T, C = x.shape
    nc = _get_nc()
    in_maps = prep_core_inputs(x, key_padding_mask, W_qkv, b_qkv, W_proj,
                               n_cores=8)
    res = run_bass_kernel_spmd(nc, in_maps, list(range(8)))
    return combine_outputs(res.results, B, T, C, b_proj)
